# revision 15
# baseline (speedup 1.0000x reference)
"""Trainium2 Bass kernel for nn_Net_66451734004145 (GRU -> "adjacency" ->
MLP -> log_softmax over the S*S pair dim).

Key structural fact: the reference's adjacency reshape (faithful torch
translation) scrambles the pairwise concat.  For p = i*S + j:
    j <  S/2 : row = [y_i, y_i]            (depends only on i)
    j >= S/2 : row = [y_{2j-S}, y_{2j-S+1}] (depends only on j)
So the MLP has only S + S/2 = 192 distinct rows per batch element: 128
"A" rows (one per i) and 64 "B" rows (one per j-64).  The dim-0
log_softmax over all S*S rows reduces to
    lse = log(64*sum_i exp(lgA_i) + 128*sum_j exp(lgB_j))
and bt cancels (constant along dim 0).  The kernel computes the GRU (the
dominant, latency-bound part: 128 sequential steps), the 192-row MLP, the
weighted lse, and expands the output via broadcast DMAs.

Sharding: data-parallel over batch B=16 across 8 cores (2 per core); the
log_softmax dim stays local, no collectives.

GRU cell (feature-major [100, 2] state, biases folded via aug ones-row,
4th negated z-gate so 1-z comes from a sigmoid):
    psum_g = gi_g + gh_g accumulated by PE (g in r, z, z')
    r,z,z' = sigmoid(psum)        (one ACT op)
    n      = tanh(ghn * r + gin)  (ACT scale/bias [P,1] fusion, per b)
    g      = z * h                (DVE, per b)
    h'     = n * z' + g           (ACT Identity scale/bias, per b)

Output NEFF layout per core: [128, 128, 4] f32 = [i, j, (b,f)]; host
reshapes to (S*S, 2, 2) and concatenates over cores along batch.
"""

import contextlib
import math

import numpy as np

import concourse.bass as bass
import concourse.mybir as mybir
import concourse.tile as tile
from concourse import bacc
from concourse.bass import ds, ts
from concourse.bass_utils import run_bass_kernel_spmd

S = 128
B = 16
IN = 64
H = 100
HID = 256
NCORES = 8
BL = B // NCORES  # 2
NR = S + S // 2  # 192 distinct MLP rows per batch element

F32 = mybir.dt.float32
AF = mybir.ActivationFunctionType
ALU = mybir.AluOpType


def bcast_free(ap, n, axis):
    """Insert a broadcast (step 0, count n) free dim at free-axis position."""
    newap = [list(d) for d in ap.ap]
    newap.insert(1 + axis, [0, n])
    return bass.AP(tensor=ap.tensor, offset=ap.offset, ap=newap)


def _emit(nc, tc):
    # ---------------- DRAM I/O ----------------
    xt = nc.dram_tensor("xt", [IN + 1, S * BL], F32, kind="ExternalInput").ap()
    h0 = nc.dram_tensor("h0", [H, BL], F32, kind="ExternalInput").ap()
    whh = nc.dram_tensor("whh", [H + 1, 4 * H], F32, kind="ExternalInput").ap()
    wih = nc.dram_tensor("wih", [IN + 1, 4 * H], F32, kind="ExternalInput").ap()
    w1ab = nc.dram_tensor("w1ab", [H + 1, HID], F32, kind="ExternalInput").ap()
    w1a = nc.dram_tensor("w1a", [H + 1, HID], F32, kind="ExternalInput").ap()
    w1b = nc.dram_tensor("w1b", [H + 1, HID], F32, kind="ExternalInput").ap()
    w2 = nc.dram_tensor("w2", [128, 2, 2, 128], F32, kind="ExternalInput").ap()
    b2v = nc.dram_tensor("b2v", [128, 2], F32, kind="ExternalInput").ap()
    w3 = nc.dram_tensor("w3", [128, 2, 10], F32, kind="ExternalInput").ap()
    b3c = nc.dram_tensor("b3c", [10, 1], F32, kind="ExternalInput").ap()
    wt = nc.dram_tensor("wt", [10, 2], F32, kind="ExternalInput").ap()
    eye2 = nc.dram_tensor("eye2", [2, 2], F32, kind="ExternalInput").ap()
    out_d = nc.dram_tensor("out", [S, S, 2 * BL], F32, kind="ExternalOutput").ap()

    with contextlib.ExitStack() as ctx:
        consts = ctx.enter_context(tc.tile_pool(name="consts", bufs=1))
        singles = ctx.enter_context(tc.tile_pool(name="singles", bufs=1))

        def load(ap_dram, shape):
            t = consts.tile(shape, F32, tag=ap_dram.tensor.name)
            nc.sync.dma_start(out=t[:], in_=ap_dram)
            return t

        xt_s = load(xt, [IN + 1, S * BL])
        whh_s = load(whh, [H + 1, 4 * H])
        wih_s = load(wih, [IN + 1, 4 * H])
        w1ab_s = load(w1ab, [H + 1, HID])
        w1a_s = load(w1a, [H + 1, HID])
        w1b_s = load(w1b, [H + 1, HID])
        w2_s = load(w2, [128, 2, 2, 128])
        b2v_s = load(b2v, [128, 2])
        w3_s = load(w3, [128, 2, 10])
        b3c_s = load(b3c, [10, 1])
        wt_s = load(wt, [10, 2])
        eye2_s = load(eye2, [2, 2])

        # Y holds [h_{-1}, h_0, ..., h_{127}] feature-major with an aug ones
        # row: Y[:, 2*(t+1)+b] = h_t for batch b.  memset everything to 1.0
        # (keeps the aug row); h0 DMA then covers rows 0:100, cols 0:2.
        Y = singles.tile([H + 1, 2 * (S + 1)], F32)
        nc.vector.memset(Y[:, :], 1.0)
        nc.sync.dma_start(out=Y[0:H, 0:BL], in_=h0)
        GIN = singles.tile([H, S * BL], F32)

        # ---------------- GRU ----------------
        with contextlib.ExitStack() as gru_ctx:
            pgi = gru_ctx.enter_context(tc.tile_pool(name="pgi", bufs=1, space="PSUM"))
            pghn = gru_ctx.enter_context(
                tc.tile_pool(name="pghn", bufs=2, space="PSUM")
            )
            rings = gru_ctx.enter_context(tc.tile_pool(name="rings", bufs=3))

            # PSUM start=True lazily zeroes a whole 2KB bank (zero region):
            # only the first matmul touching each bank may use start=True.
            # Layout [100, 4, 256]: gates r,z (bank0), z',gin (bank1); each
            # gate block is first written by its GI matmul (start on bank
            # first-toucher only), then the per-step gh matmuls accumulate
            # into already-written bytes.
            psum_gi = pgi.tile([H, 4, S * BL], F32)

            for g in range(4):
                nc.tensor.matmul(
                    psum_gi[:, g, :],
                    lhsT=wih_s[:, ts(g, H)],
                    rhs=xt_s[:],
                    start=(g % 2 == 0),
                    stop=False,
                    skip_group_check=True,
                )
            nc.scalar.activation(GIN[:], psum_gi[:, 3, :], AF.Copy)

            for t in range(S):
                hcols = Y[:, ds(2 * t, 2)]
                for g in range(3):
                    nc.tensor.matmul(
                        psum_gi[:, g, ds(2 * t, 2)],
                        lhsT=whh_s[:, ts(g, H)],
                        rhs=hcols,
                        start=False,
                        stop=True,
                        skip_group_check=True,
                    )
                ghn = pghn.tile([H, BL], F32, tag="ghn")
                nc.tensor.matmul(
                    ghn[:], lhsT=whh_s[:, ts(3, H)], rhs=hcols,
                    start=True, stop=True,
                )
                rzz = rings.tile([H, 3, BL], F32, tag="rzz")
                nc.scalar.activation(
                    rzz[:], psum_gi[:, 0:3, ds(2 * t, 2)], AF.Sigmoid
                )
                ng = rings.tile([H, BL], F32, tag="ng")
                gg = rings.tile([H, BL], F32, tag="gg")
                for b in range(BL):
                    nc.scalar.activation(
                        ng[:, ds(b, 1)], ghn[:, ds(b, 1)], AF.Tanh,
                        scale=rzz[:, 0, ds(b, 1)],
                        bias=GIN[:, ds(2 * t + b, 1)],
                    )
                    nc.vector.tensor_mul(
                        gg[:, ds(b, 1)], Y[0:H, ds(2 * t + b, 1)], rzz[:, 1, ds(b, 1)]
                    )
                    nc.scalar.activation(
                        Y[0:H, ds(2 * (t + 1) + b, 1)], ng[:, ds(b, 1)], AF.Identity,
                        scale=rzz[:, 2, ds(b, 1)],
                        bias=gg[:, ds(b, 1)],
                    )

        # ---------------- 192-row MLP + lse + output expansion ------------
        # column views of Y: all y_t for batch b / even t / odd t
        yb = Y[:, ds(2, 2 * S)].rearrange("p (i bb) -> p bb i", bb=2)
        y4 = Y[:, ds(2, 2 * S)].rearrange("p (k f) -> p f k", f=4)
        # y4[:, 2k + b, :] == y_{2j+k} columns for batch b

        with contextlib.ExitStack() as mlp_ctx:
            pmm = mlp_ctx.enter_context(tc.tile_pool(name="pmm", bufs=1, space="PSUM"))
            ptr = mlp_ctx.enter_context(tc.tile_pool(name="ptr", bufs=1, space="PSUM"))
            work = mlp_ctx.enter_context(tc.tile_pool(name="work", bufs=2))

            # [p, fc, b, row]; bank0 = cols 0:512, bank1 = 512:768.  start=True
            # only on each bank's first matmul in program order (zero-region
            # semantics); everything else relies on pending-zero overwrite /
            # accumulate-on-written-bytes.
            psAB = pmm.tile([128, 2, 2, NR], F32)
            for b in range(BL):
                for fc in range(2):
                    nc.tensor.matmul(
                        psAB[:, fc, b, ds(0, S)],
                        lhsT=w1ab_s[:, ts(fc, 128)],
                        rhs=yb[:, b, :],
                        start=(b == 0 and fc == 0), stop=False,
                        skip_group_check=True,
                    )
                    nc.tensor.matmul(
                        psAB[:, fc, b, ds(S, S // 2)],
                        lhsT=w1a_s[:, ts(fc, 128)],
                        rhs=y4[:, 0 + b, :],
                        start=(b == 0 and fc == 1), stop=False,
                        skip_group_check=True,
                    )
                    nc.tensor.matmul(
                        psAB[:, fc, b, ds(S, S // 2)],
                        lhsT=w1b_s[:, ts(fc, 128)],
                        rhs=y4[:, 2 + b, :],
                        start=False, stop=(b == 1),
                        skip_group_check=True,
                    )
            h1 = singles.tile([128, 2, 2 * NR], F32)
            nc.scalar.activation(
                h1.rearrange("p a c -> p (a c)"),
                psAB.rearrange("p a b c -> p (a b c)"),
                AF.Relu,
            )

            # mc stride padded to 512 so each matmul output stays in one bank
            ps2 = pmm.tile([128, 2, 512], F32)
            for mc in range(2):
                for kc in range(2):
                    nc.tensor.matmul(
                        ps2[:, mc, ds(0, 2 * NR)],
                        lhsT=w2_s[:, kc, mc, :],
                        rhs=h1[:, kc, :],
                        start=(kc == 0),
                        stop=(kc == 1),
                    )
            h2 = singles.tile([128, 2, 2 * NR], F32)
            for mc in range(2):
                nc.scalar.activation(
                    h2[:, mc, :], ps2[:, mc, ds(0, 2 * NR)], AF.Relu,
                    bias=b2v_s[:, ds(mc, 1)],
                )

            ps3 = pmm.tile([10, 2 * NR], F32)
            for kc in range(2):
                nc.tensor.matmul(
                    ps3[:], lhsT=w3_s[:, kc, :], rhs=h2[:, kc, :],
                    start=(kc == 0), stop=(kc == 1),
                )
            h3 = singles.tile([10, 2 * NR], F32)
            nc.scalar.activation(h3[:], ps3[:], AF.Relu, bias=b3c_s[:, ds(0, 1)])

            ps4 = pmm.tile([2, 2 * NR], F32)  # logits [f, (b, row)]
            nc.tensor.matmul(ps4[:], lhsT=wt_s[:], rhs=h3[:], start=True, stop=True)

            # weighted lse over dim 0: log(64*sum exp lgA + 128*sum exp lgB)
            sA = singles.tile([2, BL], F32)
            sB = singles.tile([2, BL], F32)
            scr = singles.tile([2, 2 * NR], F32)
            for b in range(BL):
                nc.scalar.activation(
                    scr[:, ds(b * NR, S)], ps4[:, ds(b * NR, S)], AF.Exp,
                    accum_out=sA[:, ds(b, 1)],
                )
                nc.scalar.activation(
                    scr[:, ds(b * NR + S, S // 2)], ps4[:, ds(b * NR + S, S // 2)],
                    AF.Exp,
                    accum_out=sB[:, ds(b, 1)],
                )
            # B rows are counted 128x vs A's 64x: s = sA + 2*sB
            ssum = singles.tile([2, BL], F32)
            nc.vector.scalar_tensor_tensor(
                ssum[:], sB[:], 2.0, sA[:], op0=ALU.mult, op1=ALU.add
            )
            lse = singles.tile([2, BL], F32)
            nc.scalar.activation(lse[:], ssum[:], AF.Ln, scale=64.0)
            nlse = singles.tile([2, BL], F32)
            nc.vector.tensor_scalar_mul(nlse[:], lse[:], -1.0)

            lgAT = singles.tile([128, 2 * BL], F32)  # [i, (b, f)]
            lgBT = singles.tile([64, 2 * BL], F32)  # [jj, (b, f)]
            for b in range(BL):
                lg = work.tile([2, NR], F32, tag="lg")
                nc.scalar.activation(
                    lg[:], ps4[:, ds(b * NR, NR)], AF.Identity, bias=nlse[:, ds(b, 1)]
                )
                pA = ptr.tile([128, 2], F32, tag="pA")
                nc.tensor.transpose(pA[:], lg[:, ds(0, S)], eye2_s[:])
                nc.vector.tensor_copy(lgAT[:, ds(2 * b, 2)], pA[:])
                pB = ptr.tile([64, 2], F32, tag="pB")
                nc.tensor.transpose(pB[:], lg[:, ds(S, S // 2)], eye2_s[:])
                nc.vector.tensor_copy(lgBT[:, ds(2 * b, 2)], pB[:])

            # output expansion via broadcast DMAs
            nc.sync.dma_start(
                out=out_d[:, 0:64, :], in_=bcast_free(lgAT[:, :], 64, 0)
            )
            nc.sync.dma_start(
                out=out_d[:, 64:128, :].rearrange("i j c -> j i c"),
                in_=bcast_free(lgBT[:, :], S, 0),
            )

        import os
        if os.environ.get("KERNEL_DEBUG_Y"):
            ydbg = nc.dram_tensor(
                "ydbg", [H + 1, 2 * (S + 1)], F32, kind="ExternalOutput"
            ).ap()
            nc.sync.dma_start(out=ydbg, in_=Y[:, :])


def build_nc():
    nc = bacc.Bacc(
        "TRN2",
        target_bir_lowering=False,
        debug=False,
        enable_asserts=False,
        num_devices=NCORES,
    )
    with tile.TileContext(nc) as tc:
        _emit(nc, tc)
    nc.compile()
    return nc


def prep_weights(W_ih, W_hh, b_ih, b_hh, W1, b1, W2, b2, W3, b3, Wt, bt):
    """Host-side weight preprocessing shared by all cores."""
    f = np.float32
    W_ih, W_hh = f(W_ih), f(W_hh)
    b_ih, b_hh = f(b_ih), f(b_hh)
    W1, b1, W2, b2 = f(W1), f(b1), f(W2), f(b2)
    W3, b3, Wt = f(W3), f(b3), f(Wt)

    def gate(W, bvec, g, sign=1.0):
        blk = np.concatenate(
            [W[g * H : (g + 1) * H].T, bvec[g * H : (g + 1) * H][None, :]], axis=0
        )
        return sign * blk

    whh = np.concatenate(
        [gate(W_hh, b_hh, 0), gate(W_hh, b_hh, 1),
         gate(W_hh, b_hh, 1, -1.0), gate(W_hh, b_hh, 2)], axis=1
    )
    wih = np.concatenate(
        [gate(W_ih, b_ih, 0), gate(W_ih, b_ih, 1),
         gate(W_ih, b_ih, 1, -1.0), gate(W_ih, b_ih, 2)], axis=1
    )
    W1a, W1b = W1[:, :H], W1[:, H:]
    zrow = np.zeros((1, HID), np.float32)
    w1ab = np.concatenate([(W1a + W1b).T, b1[None, :]], axis=0)
    w1a_p = np.concatenate([W1a.T, b1[None, :]], axis=0)
    w1b_p = np.concatenate([W1b.T, zrow], axis=0)
    w2 = np.ascontiguousarray(W2.reshape(2, 128, 2, 128).transpose(3, 2, 0, 1))
    b2v = np.ascontiguousarray(b2.reshape(2, 128).T)
    w3 = np.ascontiguousarray(
        W3.reshape(10, 2, 128).transpose(2, 1, 0)
    )  # [p, kc, m]
    return {
        "whh": np.ascontiguousarray(whh),
        "wih": np.ascontiguousarray(wih),
        "w1ab": np.ascontiguousarray(w1ab),
        "w1a": np.ascontiguousarray(w1a_p),
        "w1b": np.ascontiguousarray(w1b_p),
        "w2": w2,
        "b2v": b2v,
        "w3": w3,
        "b3c": np.ascontiguousarray(b3[:, None]),
        "wt": np.ascontiguousarray(Wt.T),
        "eye2": np.eye(2, dtype=np.float32),
    }


def make_in_maps(x, hidden, weights):
    x = np.asarray(x, np.float32)
    hidden = np.asarray(hidden, np.float32)
    in_maps = []
    for c in range(NCORES):
        b0 = c * BL
        xs = x[:, b0 : b0 + BL, :]
        xtc = np.concatenate(
            [xs.transpose(2, 0, 1).reshape(IN, S * BL),
             np.ones((1, S * BL), np.float32)], axis=0
        )
        m = dict(weights)
        m["xt"] = np.ascontiguousarray(xtc)
        m["h0"] = np.ascontiguousarray(hidden[0, b0 : b0 + BL, :].T)
        in_maps.append(m)
    return in_maps


def postprocess(results):
    outs = []
    for r in results:
        a = r["out"].reshape(S * S, BL, 2)
        outs.append(np.ascontiguousarray(a))
    return np.concatenate(outs, axis=1)


_NC_CACHE = {}


def get_nc():
    if "nc" not in _NC_CACHE:
        _NC_CACHE["nc"] = build_nc()
    return _NC_CACHE["nc"]


LAST_RESULTS = None


def kernel(x, hidden, W_ih, W_hh, b_ih, b_hh, W1, b1, W2, b2, W3, b3, Wt, bt,
           _run_kwargs=None):
    global LAST_RESULTS
    weights = prep_weights(W_ih, W_hh, b_ih, b_hh, W1, b1, W2, b2, W3, b3, Wt, bt)
    in_maps = make_in_maps(x, hidden, weights)
    nc = get_nc()
    res = run_bass_kernel_spmd(
        nc, in_maps, core_ids=list(range(NCORES)), **(_run_kwargs or {})
    )
    LAST_RESULTS = res
    return postprocess(res.results)


# revision 28
# speedup vs baseline: 1.2394x; 1.2394x over previous
"""Trainium2 Bass kernel for nn_Net_66451734004145 (GRU -> "adjacency" ->
MLP -> log_softmax over the S*S pair dim).

Key structural fact: the reference's adjacency reshape (faithful torch
translation) scrambles the pairwise concat.  For p = i*S + j:
    j <  S/2 : row = [y_i, y_i]            (depends only on i)
    j >= S/2 : row = [y_{2j-S}, y_{2j-S+1}] (depends only on j)
So the MLP has only S + S/2 = 192 distinct rows per batch element: 128
"A" rows (one per i) and 64 "B" rows (one per j-64).  The dim-0
log_softmax over all S*S rows reduces to
    lse = log(64*sum_i exp(lgA_i) + 128*sum_j exp(lgB_j))
and bt cancels (constant along dim 0).  The kernel computes the GRU (the
dominant, latency-bound part: 128 sequential steps), the 192-row MLP, the
weighted lse, and expands the output via broadcast DMAs.

Sharding: data-parallel over batch B=16 across 8 cores (2 per core); the
log_softmax dim stays local, no collectives.

GRU cell (feature-major [100, 2] state, biases folded via aug ones-row,
4th negated z-gate so 1-z comes from a sigmoid):
    psum_g = gi_g + gh_g accumulated by PE (g in r, z, z')
    r,z,z' = sigmoid(psum)        (one ACT op)
    n      = tanh(ghn * r + gin)  (ACT scale/bias [P,1] fusion, per b)
    g      = z * h                (DVE, per b)
    h'     = n * z' + g           (ACT Identity scale/bias, per b)

Output NEFF layout per core: [128, 128, 4] f32 = [i, j, (b,f)]; host
reshapes to (S*S, 2, 2) and concatenates over cores along batch.
"""

import contextlib
import math

import numpy as np

import concourse.bass as bass
import concourse.mybir as mybir
import concourse.tile as tile
from concourse import bacc
from concourse.bass import ds, ts
from concourse.bass_utils import run_bass_kernel_spmd

S = 128
B = 16
IN = 64
H = 100
HID = 256
NCORES = 8
BL = B // NCORES  # 2
NR = S + S // 2  # 192 distinct MLP rows per batch element

F32 = mybir.dt.float32
F32R = mybir.dt.float32r
AF = mybir.ActivationFunctionType
ALU = mybir.AluOpType


def bcast_free(ap, n, axis):
    """Insert a broadcast (step 0, count n) free dim at free-axis position."""
    newap = [list(d) for d in ap.ap]
    newap.insert(1 + axis, [0, n])
    return bass.AP(tensor=ap.tensor, offset=ap.offset, ap=newap)


def _emit(nc, tc):
    # ---------------- DRAM I/O ----------------
    xt = nc.dram_tensor("xt", [IN + 1, S * BL], F32, kind="ExternalInput").ap()
    y0 = nc.dram_tensor("y0", [H + 1, 2 * (S + 1)], F32R, kind="ExternalInput").ap()
    whh = nc.dram_tensor("whh", [H + 1, 3 * H], F32R, kind="ExternalInput").ap()
    wih = nc.dram_tensor("wih", [IN + 1, 3 * H], F32, kind="ExternalInput").ap()
    w1ab = nc.dram_tensor("w1ab", [H + 1, HID], F32R, kind="ExternalInput").ap()
    w1a = nc.dram_tensor("w1a", [H + 1, HID], F32R, kind="ExternalInput").ap()
    w1b = nc.dram_tensor("w1b", [H + 1, HID], F32R, kind="ExternalInput").ap()
    w2 = nc.dram_tensor("w2", [128, 2, 2, 128], F32, kind="ExternalInput").ap()
    b2v = nc.dram_tensor("b2v", [128, 2], F32, kind="ExternalInput").ap()
    w3 = nc.dram_tensor("w3", [128, 2, 10], F32, kind="ExternalInput").ap()
    b3c = nc.dram_tensor("b3c", [10, 1], F32, kind="ExternalInput").ap()
    wt = nc.dram_tensor("wt", [10, 2], F32, kind="ExternalInput").ap()
    eye2 = nc.dram_tensor("eye2", [2, 2], F32, kind="ExternalInput").ap()
    out_d = nc.dram_tensor("out", [S, S, 2 * BL], F32, kind="ExternalOutput").ap()

    with contextlib.ExitStack() as ctx:
        consts = ctx.enter_context(tc.tile_pool(name="consts", bufs=1))
        singles = ctx.enter_context(tc.tile_pool(name="singles", bufs=1))

        def load(ap_dram, shape):
            t = consts.tile(shape, ap_dram.tensor.dtype, tag=ap_dram.tensor.name)
            nc.sync.dma_start(out=t[:], in_=ap_dram)
            return t

        xt_s = load(xt, [IN + 1, S * BL])
        whh_s = load(whh, [H + 1, 3 * H])
        wih_s = load(wih, [IN + 1, 3 * H])
        w1ab_s = load(w1ab, [H + 1, HID])
        w1a_s = load(w1a, [H + 1, HID])
        w1b_s = load(w1b, [H + 1, HID])
        w2_s = load(w2, [128, 2, 2, 128])
        b2v_s = load(b2v, [128, 2])
        w3_s = load(w3, [128, 2, 10])
        b3c_s = load(b3c, [10, 1])
        wt_s = load(wt, [10, 2])
        eye2_s = load(eye2, [2, 2])

        # Y holds [h_{-1}, h_0, ..., h_{127}] feature-major with an aug ones
        # row: Y[:, 2*(t+1)+b] = h_t for batch b.  memset everything to 1.0
        # (keeps the aug row); h0 DMA then covers rows 0:100, cols 0:2.
        # Y is float32r: the recurrent matmuls and the MLP's A/B matmuls
        # consume it on the PE; non-PE readers bitcast to f32.  Initial
        # content (aug ones row + h0 in cols 0:2) comes in via one DMA.
        Y = singles.tile([H + 1, 2 * (S + 1)], F32R)
        nc.sync.dma_start(out=Y[:, :], in_=y0)
        GIN = singles.tile([H, S * BL], F32)

        # ---------------- GRU ----------------
        with contextlib.ExitStack() as gru_ctx:
            pgi = gru_ctx.enter_context(tc.tile_pool(name="pgi", bufs=1, space="PSUM"))
            pghn = gru_ctx.enter_context(
                tc.tile_pool(name="pghn", bufs=2, space="PSUM")
            )
            rings = gru_ctx.enter_context(tc.tile_pool(name="rings", bufs=3))

            # PSUM start=True lazily zeroes a whole 2KB bank (zero region):
            # only the first matmul touching each bank may use start=True.
            # Layout [100, 3, 256]: gates r,z' (bank0), gin (bank1); each
            # gate block is first written by its GI matmul (start on bank
            # first-toucher only), then the per-step gh matmuls accumulate
            # into already-written bytes.
            # Cell: h' = z'*(n - h) + h with z' = sigmoid(-(i_z + h_z))
            # (z-gate weights negated on host), so no z gate is computed.
            psum_gi = pgi.tile([H, 3, S * BL], F32)

            for g in range(3):
                nc.tensor.matmul(
                    psum_gi[:, g, :],
                    lhsT=wih_s[:, ts(g, H)],
                    rhs=xt_s[:],
                    start=(g % 2 == 0),
                    stop=False,
                    skip_group_check=True,
                )
            nc.scalar.activation(GIN[:], psum_gi[:, 2, :], AF.Copy)

            for t in range(S):
                hcols = Y[:, ds(2 * t, 2)]
                for g in range(2):
                    nc.tensor.matmul(
                        psum_gi[:, g, ds(2 * t, 2)],
                        lhsT=whh_s[:, ts(g, H)],
                        rhs=hcols,
                        start=False,
                        stop=True,
                        skip_group_check=True,
                    )
                ghn = pghn.tile([H, BL], F32, tag="ghn")
                nc.tensor.matmul(
                    ghn[:], lhsT=whh_s[:, ts(2, H)], rhs=hcols,
                    start=True, stop=True,
                )
                rzp = rings.tile([H, 2, BL], F32, tag="rzp")
                nc.scalar.activation(
                    rzp[:], psum_gi[:, 0:2, ds(2 * t, 2)], AF.Sigmoid
                )
                ng = rings.tile([H, BL], F32, tag="ng")
                ee = rings.tile([H, BL], F32, tag="ee")
                for b in range(BL):
                    nc.scalar.activation(
                        ng[:, ds(b, 1)], ghn[:, ds(b, 1)], AF.Tanh,
                        scale=rzp[:, 0, ds(b, 1)],
                        bias=GIN[:, ds(2 * t + b, 1)],
                    )
                # e = n - h (both batches in one op)
                nc.vector.tensor_sub(
                    ee[:], ng[:], Y[0:H, ds(2 * t, 2)].bitcast(F32)
                )
                for b in range(BL):
                    # h' = e*z' + h  (out stays f32r for the next matmul)
                    nc.vector.scalar_tensor_tensor(
                        Y[0:H, ds(2 * (t + 1) + b, 1)],
                        ee[:, ds(b, 1)],
                        rzp[:, 1, ds(b, 1)],
                        Y[0:H, ds(2 * t + b, 1)].bitcast(F32),
                        op0=ALU.mult,
                        op1=ALU.add,
                    )

        # ---------------- 192-row MLP + lse + output expansion ------------
        # column views of Y: all y_t for batch b / even t / odd t
        yb = Y[:, ds(2, 2 * S)].rearrange("p (i bb) -> p bb i", bb=2)
        y4 = Y[:, ds(2, 2 * S)].rearrange("p (k f) -> p f k", f=4)
        # y4[:, 2k + b, :] == y_{2j+k} columns for batch b

        with contextlib.ExitStack() as mlp_ctx:
            pmm = mlp_ctx.enter_context(tc.tile_pool(name="pmm", bufs=1, space="PSUM"))
            ptr = mlp_ctx.enter_context(tc.tile_pool(name="ptr", bufs=1, space="PSUM"))
            work = mlp_ctx.enter_context(tc.tile_pool(name="work", bufs=2))

            # [p, fc, b, row]; bank0 = cols 0:512, bank1 = 512:768.  start=True
            # only on each bank's first matmul in program order (zero-region
            # semantics); everything else relies on pending-zero overwrite /
            # accumulate-on-written-bytes.
            psAB = pmm.tile([128, 2, 2, NR], F32)
            for b in range(BL):
                for fc in range(2):
                    nc.tensor.matmul(
                        psAB[:, fc, b, ds(0, S)],
                        lhsT=w1ab_s[:, ts(fc, 128)],
                        rhs=yb[:, b, :],
                        start=(b == 0 and fc == 0), stop=False,
                        skip_group_check=True,
                    )
                    nc.tensor.matmul(
                        psAB[:, fc, b, ds(S, S // 2)],
                        lhsT=w1a_s[:, ts(fc, 128)],
                        rhs=y4[:, 0 + b, :],
                        start=(b == 0 and fc == 1), stop=False,
                        skip_group_check=True,
                    )
                    nc.tensor.matmul(
                        psAB[:, fc, b, ds(S, S // 2)],
                        lhsT=w1b_s[:, ts(fc, 128)],
                        rhs=y4[:, 2 + b, :],
                        start=False, stop=(b == 1),
                        skip_group_check=True,
                    )
            h1 = singles.tile([128, 2, 2 * NR], F32)
            nc.scalar.activation(
                h1.rearrange("p a c -> p (a c)"),
                psAB.rearrange("p a b c -> p (a b c)"),
                AF.Relu,
            )

            # mc stride padded to 512 so each matmul output stays in one bank
            ps2 = pmm.tile([128, 2, 512], F32)
            for mc in range(2):
                for kc in range(2):
                    nc.tensor.matmul(
                        ps2[:, mc, ds(0, 2 * NR)],
                        lhsT=w2_s[:, kc, mc, :],
                        rhs=h1[:, kc, :],
                        start=(kc == 0),
                        stop=(kc == 1),
                    )
            h2 = singles.tile([128, 2, 2 * NR], F32)
            for mc in range(2):
                nc.scalar.activation(
                    h2[:, mc, :], ps2[:, mc, ds(0, 2 * NR)], AF.Relu,
                    bias=b2v_s[:, ds(mc, 1)],
                )

            ps3 = pmm.tile([10, 2 * NR], F32)
            for kc in range(2):
                nc.tensor.matmul(
                    ps3[:], lhsT=w3_s[:, kc, :], rhs=h2[:, kc, :],
                    start=(kc == 0), stop=(kc == 1),
                )
            h3 = singles.tile([10, 2 * NR], F32)
            nc.scalar.activation(h3[:], ps3[:], AF.Relu, bias=b3c_s[:, ds(0, 1)])

            ps4 = pmm.tile([2, 2 * NR], F32)  # logits [f, (b, row)]
            nc.tensor.matmul(ps4[:], lhsT=wt_s[:], rhs=h3[:], start=True, stop=True)

            # weighted lse over dim 0: log(64*sum exp lgA + 128*sum exp lgB)
            sA = singles.tile([2, BL], F32)
            sB = singles.tile([2, BL], F32)
            scr = singles.tile([2, 2 * NR], F32)
            for b in range(BL):
                nc.scalar.activation(
                    scr[:, ds(b * NR, S)], ps4[:, ds(b * NR, S)], AF.Exp,
                    accum_out=sA[:, ds(b, 1)],
                )
                nc.scalar.activation(
                    scr[:, ds(b * NR + S, S // 2)], ps4[:, ds(b * NR + S, S // 2)],
                    AF.Exp,
                    accum_out=sB[:, ds(b, 1)],
                )
            # B rows are counted 128x vs A's 64x: s = sA + 2*sB
            ssum = singles.tile([2, BL], F32)
            nc.vector.scalar_tensor_tensor(
                ssum[:], sB[:], 2.0, sA[:], op0=ALU.mult, op1=ALU.add
            )
            lse = singles.tile([2, BL], F32)
            nc.scalar.activation(lse[:], ssum[:], AF.Ln, scale=64.0)
            nlse = singles.tile([2, BL], F32)
            nc.vector.tensor_scalar_mul(nlse[:], lse[:], -1.0)

            lgAT = singles.tile([128, 2 * BL], F32)  # [i, (b, f)]
            lgBT = singles.tile([64, 2 * BL], F32)  # [jj, (b, f)]
            for b in range(BL):
                lg = work.tile([2, NR], F32, tag="lg")
                nc.scalar.activation(
                    lg[:], ps4[:, ds(b * NR, NR)], AF.Identity, bias=nlse[:, ds(b, 1)]
                )
                pA = ptr.tile([128, 2], F32, tag="pA")
                nc.tensor.transpose(pA[:], lg[:, ds(0, S)], eye2_s[:])
                nc.vector.tensor_copy(lgAT[:, ds(2 * b, 2)], pA[:])
                pB = ptr.tile([64, 2], F32, tag="pB")
                nc.tensor.transpose(pB[:], lg[:, ds(S, S // 2)], eye2_s[:])
                nc.vector.tensor_copy(lgBT[:, ds(2 * b, 2)], pB[:])

            # output expansion via broadcast DMAs
            nc.sync.dma_start(
                out=out_d[:, 0:64, :], in_=bcast_free(lgAT[:, :], 64, 0)
            )
            nc.sync.dma_start(
                out=out_d[:, 64:128, :].rearrange("i j c -> j i c"),
                in_=bcast_free(lgBT[:, :], S, 0),
            )

        import os
        if os.environ.get("KERNEL_DEBUG_Y"):
            ydbg = nc.dram_tensor(
                "ydbg", [H + 1, 2 * (S + 1)], F32, kind="ExternalOutput"
            ).ap()
            nc.sync.dma_start(out=ydbg, in_=Y[:, :])


def build_nc():
    nc = bacc.Bacc(
        "TRN2",
        target_bir_lowering=False,
        debug=False,
        enable_asserts=False,
        num_devices=NCORES,
    )
    with tile.TileContext(nc) as tc:
        _emit(nc, tc)
    nc.compile()
    return nc


def prep_weights(W_ih, W_hh, b_ih, b_hh, W1, b1, W2, b2, W3, b3, Wt, bt):
    """Host-side weight preprocessing shared by all cores."""
    f = np.float32
    W_ih, W_hh = f(W_ih), f(W_hh)
    b_ih, b_hh = f(b_ih), f(b_hh)
    W1, b1, W2, b2 = f(W1), f(b1), f(W2), f(b2)
    W3, b3, Wt = f(W3), f(b3), f(Wt)

    def gate(W, bvec, g, sign=1.0):
        blk = np.concatenate(
            [W[g * H : (g + 1) * H].T, bvec[g * H : (g + 1) * H][None, :]], axis=0
        )
        return sign * blk

    # gate blocks [r, z'(= -z), n]: z' weights negated so sigmoid gives 1-z
    whh = np.concatenate(
        [gate(W_hh, b_hh, 0), gate(W_hh, b_hh, 1, -1.0), gate(W_hh, b_hh, 2)],
        axis=1,
    )
    wih = np.concatenate(
        [gate(W_ih, b_ih, 0), gate(W_ih, b_ih, 1, -1.0), gate(W_ih, b_ih, 2)],
        axis=1,
    )
    W1a, W1b = W1[:, :H], W1[:, H:]
    zrow = np.zeros((1, HID), np.float32)
    w1ab = np.concatenate([(W1a + W1b).T, b1[None, :]], axis=0)
    w1a_p = np.concatenate([W1a.T, b1[None, :]], axis=0)
    w1b_p = np.concatenate([W1b.T, zrow], axis=0)
    w2 = np.ascontiguousarray(W2.reshape(2, 128, 2, 128).transpose(3, 2, 0, 1))
    b2v = np.ascontiguousarray(b2.reshape(2, 128).T)
    w3 = np.ascontiguousarray(
        W3.reshape(10, 2, 128).transpose(2, 1, 0)
    )  # [p, kc, m]
    return {
        "whh": np.ascontiguousarray(whh),
        "wih": np.ascontiguousarray(wih),
        "w1ab": np.ascontiguousarray(w1ab),
        "w1a": np.ascontiguousarray(w1a_p),
        "w1b": np.ascontiguousarray(w1b_p),
        "w2": w2,
        "b2v": b2v,
        "w3": w3,
        "b3c": np.ascontiguousarray(b3[:, None]),
        "wt": np.ascontiguousarray(Wt.T),
        "eye2": np.eye(2, dtype=np.float32),
    }


def make_in_maps(x, hidden, weights):
    x = np.asarray(x, np.float32)
    hidden = np.asarray(hidden, np.float32)
    in_maps = []
    for c in range(NCORES):
        b0 = c * BL
        xs = x[:, b0 : b0 + BL, :]
        xtc = np.concatenate(
            [xs.transpose(2, 0, 1).reshape(IN, S * BL),
             np.ones((1, S * BL), np.float32)], axis=0
        )
        m = dict(weights)
        m["xt"] = np.ascontiguousarray(xtc)
        y0 = np.ones((H + 1, 2 * (S + 1)), np.float32)
        y0[0:H, 0:BL] = hidden[0, b0 : b0 + BL, :].T
        m["y0"] = y0
        in_maps.append(m)
    return in_maps


def postprocess(results):
    outs = []
    for r in results:
        a = r["out"].reshape(S * S, BL, 2)
        outs.append(np.ascontiguousarray(a))
    return np.concatenate(outs, axis=1)


_NC_CACHE = {}


def get_nc():
    if "nc" not in _NC_CACHE:
        _NC_CACHE["nc"] = build_nc()
    return _NC_CACHE["nc"]


LAST_RESULTS = None


def kernel(x, hidden, W_ih, W_hh, b_ih, b_hh, W1, b1, W2, b2, W3, b3, Wt, bt,
           _run_kwargs=None):
    global LAST_RESULTS
    weights = prep_weights(W_ih, W_hh, b_ih, b_hh, W1, b1, W2, b2, W3, b3, Wt, bt)
    in_maps = make_in_maps(x, hidden, weights)
    nc = get_nc()
    res = run_bass_kernel_spmd(
        nc, in_maps, core_ids=list(range(NCORES)), **(_run_kwargs or {})
    )
    LAST_RESULTS = res
    return postprocess(res.results)


# revision 35
# speedup vs baseline: 1.2684x; 1.0234x over previous
"""Trainium2 Bass kernel for nn_Net_66451734004145 (GRU -> "adjacency" ->
MLP -> log_softmax over the S*S pair dim).

Key structural fact: the reference's adjacency reshape (faithful torch
translation) scrambles the pairwise concat.  For p = i*S + j:
    j <  S/2 : row = [y_i, y_i]            (depends only on i)
    j >= S/2 : row = [y_{2j-S}, y_{2j-S+1}] (depends only on j)
So the MLP has only S + S/2 = 192 distinct rows per batch element: 128
"A" rows (one per i) and 64 "B" rows (one per j-64).  The dim-0
log_softmax over all S*S rows reduces to
    lse = log(64*sum_i exp(lgA_i) + 128*sum_j exp(lgB_j))
and bt cancels (constant along dim 0).  The kernel computes the GRU (the
dominant, latency-bound part: 128 sequential steps), the 192-row MLP, the
weighted lse, and expands the output via broadcast DMAs.

Sharding: data-parallel over batch B=16 across 8 cores (2 per core); the
log_softmax dim stays local, no collectives.

GRU cell (feature-major [100, 2] state, biases folded via aug ones-row,
4th negated z-gate so 1-z comes from a sigmoid):
    psum_g = gi_g + gh_g accumulated by PE (g in r, z, z')
    r,z,z' = sigmoid(psum)        (one ACT op)
    n      = tanh(ghn * r + gin)  (ACT scale/bias [P,1] fusion, per b)
    g      = z * h                (DVE, per b)
    h'     = n * z' + g           (ACT Identity scale/bias, per b)

Output NEFF layout per core: [128, 128, 4] f32 = [i, j, (b,f)]; host
reshapes to (S*S, 2, 2) and concatenates over cores along batch.
"""

import contextlib
import math

import numpy as np

import concourse.bass as bass
import concourse.mybir as mybir
import concourse.tile as tile
from concourse import bacc
from concourse.bass import ds, ts
from concourse.bass_utils import run_bass_kernel_spmd

S = 128
B = 16
IN = 64
H = 100
HID = 256
NCORES = 8
BL = B // NCORES  # 2
NR = S + S // 2  # 192 distinct MLP rows per batch element

F32 = mybir.dt.float32
F32R = mybir.dt.float32r
AF = mybir.ActivationFunctionType
ALU = mybir.AluOpType


def bcast_free(ap, n, axis):
    """Insert a broadcast (step 0, count n) free dim at free-axis position."""
    newap = [list(d) for d in ap.ap]
    newap.insert(1 + axis, [0, n])
    return bass.AP(tensor=ap.tensor, offset=ap.offset, ap=newap)


def _emit(nc, tc):
    # ---------------- DRAM I/O ----------------
    xt = nc.dram_tensor("xt", [IN + 1, S * BL], F32, kind="ExternalInput").ap()
    y0 = nc.dram_tensor("y0", [H + 1, 2 * (S + 1)], F32R, kind="ExternalInput").ap()
    whh = nc.dram_tensor("whh", [H + 1, 3 * H], F32R, kind="ExternalInput").ap()
    wih = nc.dram_tensor("wih", [IN + 1, 3 * H], F32, kind="ExternalInput").ap()
    w1ab = nc.dram_tensor("w1ab", [H + 1, HID], F32R, kind="ExternalInput").ap()
    w1a = nc.dram_tensor("w1a", [H + 1, HID], F32R, kind="ExternalInput").ap()
    w1b = nc.dram_tensor("w1b", [H + 1, HID], F32R, kind="ExternalInput").ap()
    w2 = nc.dram_tensor("w2", [128, 2, 2, 128], F32, kind="ExternalInput").ap()
    b2v = nc.dram_tensor("b2v", [128, 2], F32, kind="ExternalInput").ap()
    w3 = nc.dram_tensor("w3", [128, 2, 10], F32, kind="ExternalInput").ap()
    b3c = nc.dram_tensor("b3c", [10, 1], F32, kind="ExternalInput").ap()
    wt = nc.dram_tensor("wt", [10, 2], F32, kind="ExternalInput").ap()
    eye2 = nc.dram_tensor("eye2", [2, 2], F32, kind="ExternalInput").ap()
    onesrow = nc.dram_tensor("onesrow", [1, 128], F32, kind="ExternalInput").ap()
    out_d = nc.dram_tensor("out", [S, S, 2 * BL], F32, kind="ExternalOutput").ap()

    with contextlib.ExitStack() as ctx:
        consts = ctx.enter_context(tc.tile_pool(name="consts", bufs=1))
        singles = ctx.enter_context(tc.tile_pool(name="singles", bufs=1))

        def load(ap_dram, shape, eng=None):
            t = consts.tile(shape, ap_dram.tensor.dtype, tag=ap_dram.tensor.name)
            (eng or nc.sync).dma_start(out=t[:], in_=ap_dram)
            return t

        # GRU-critical loads on the sync queue; the rest spread over other
        # engines' DMA queues so they run in parallel / off the critical path.
        xt_s = load(xt, [IN + 1, S * BL])
        wih_s = load(wih, [IN + 1, 3 * H])
        whh_s = load(whh, [H + 1, 3 * H])
        w1ab_s = load(w1ab, [H + 1, HID], nc.gpsimd)
        w1a_s = load(w1a, [H + 1, HID], nc.gpsimd)
        w1b_s = load(w1b, [H + 1, HID], nc.gpsimd)
        w2_s = load(w2, [128, 2, 2, 128], nc.scalar)
        b2v_s = load(b2v, [128, 2], nc.scalar)
        w3_s = load(w3, [128, 2, 10], nc.scalar)
        b3c_s = load(b3c, [10, 1], nc.scalar)
        wt_s = load(wt, [10, 2], nc.scalar)
        eye2_s = load(eye2, [2, 2], nc.gpsimd)
        ones_r = load(onesrow, [1, 128], nc.gpsimd)

        # Y holds [h_{-1}, h_0, ..., h_{127}] feature-major with an aug ones
        # row: Y[:, 2*(t+1)+b] = h_t for batch b.  memset everything to 1.0
        # (keeps the aug row); h0 DMA then covers rows 0:100, cols 0:2.
        # Y is float32r: the recurrent matmuls and the MLP's A/B matmuls
        # consume it on the PE; non-PE readers bitcast to f32.  Initial
        # content (aug ones row + h0 in cols 0:2) comes in via one DMA.
        Y = singles.tile([H + 1, 2 * (S + 1)], F32R)
        nc.sync.dma_start(out=Y[:, :], in_=y0)
        GIN = singles.tile([H, S * BL], F32)

        # ---------------- GRU ----------------
        with contextlib.ExitStack() as gru_ctx:
            pgi = gru_ctx.enter_context(tc.tile_pool(name="pgi", bufs=1, space="PSUM"))
            pghn = gru_ctx.enter_context(
                tc.tile_pool(name="pghn", bufs=2, space="PSUM")
            )
            rings = gru_ctx.enter_context(tc.tile_pool(name="rings", bufs=3))

            # PSUM start=True lazily zeroes a whole 2KB bank (zero region):
            # only the first matmul touching each bank may use start=True.
            # Layout [100, 3, 256]: gates r,z' (bank0), gin (bank1); each
            # gate block is first written by its GI matmul (start on bank
            # first-toucher only), then the per-step gh matmuls accumulate
            # into already-written bytes.
            # Cell: h' = z'*(n - h) + h with z' = sigmoid(-(i_z + h_z))
            # (z-gate weights negated on host), so no z gate is computed.
            psum_gi = pgi.tile([H, 3, S * BL], F32)

            for g in range(3):
                nc.tensor.matmul(
                    psum_gi[:, g, :],
                    lhsT=wih_s[:, ts(g, H)],
                    rhs=xt_s[:],
                    start=(g % 2 == 0),
                    stop=False,
                    skip_group_check=True,
                )
            nc.scalar.activation(GIN[:], psum_gi[:, 2, :], AF.Copy)

            for t in range(S):
                hcols = Y[:, ds(2 * t, 2)]
                for g in range(2):
                    nc.tensor.matmul(
                        psum_gi[:, g, ds(2 * t, 2)],
                        lhsT=whh_s[:, ts(g, H)],
                        rhs=hcols,
                        start=False,
                        stop=True,
                        skip_group_check=True,
                    )
                ghn = pghn.tile([H, BL], F32, tag="ghn")
                nc.tensor.matmul(
                    ghn[:], lhsT=whh_s[:, ts(2, H)], rhs=hcols,
                    start=True, stop=True,
                )
                rzp = rings.tile([H, 2, BL], F32, tag="rzp")
                nc.scalar.activation(
                    rzp[:], psum_gi[:, 0:2, ds(2 * t, 2)], AF.Sigmoid
                )
                ng = rings.tile([H, BL], F32, tag="ng")
                ee = rings.tile([H, BL], F32, tag="ee")
                for b in range(BL):
                    nc.scalar.activation(
                        ng[:, ds(b, 1)], ghn[:, ds(b, 1)], AF.Tanh,
                        scale=rzp[:, 0, ds(b, 1)],
                        bias=GIN[:, ds(2 * t + b, 1)],
                    )
                # h' = z'*(n - h) + h, all [100, 2] merged-batch DVE ops;
                # the final add writes f32r for the next step's matmul.
                mm_ = rings.tile([H, BL], F32, tag="mm")
                nc.vector.tensor_sub(
                    ee[:], ng[:], Y[0:H, ds(2 * t, 2)].bitcast(F32)
                )
                nc.vector.tensor_mul(mm_[:], ee[:], rzp[:, 1, :])
                nc.vector.tensor_add(
                    Y[0:H, ds(2 * (t + 1), 2)],
                    mm_[:],
                    Y[0:H, ds(2 * t, 2)].bitcast(F32),
                )

        # ---------------- 192-row MLP + lse + output expansion ------------
        # column views of Y: all y_t for batch b / even t / odd t
        yb = Y[:, ds(2, 2 * S)].rearrange("p (i bb) -> p bb i", bb=2)
        y4 = Y[:, ds(2, 2 * S)].rearrange("p (k f) -> p f k", f=4)
        # y4[:, 2k + b, :] == y_{2j+k} columns for batch b

        with contextlib.ExitStack() as mlp_ctx:
            pmm = mlp_ctx.enter_context(tc.tile_pool(name="pmm", bufs=1, space="PSUM"))
            ptr = mlp_ctx.enter_context(tc.tile_pool(name="ptr", bufs=1, space="PSUM"))
            work = mlp_ctx.enter_context(tc.tile_pool(name="work", bufs=2))

            # [p, fc, b, row]; bank0 = cols 0:512, bank1 = 512:768.  start=True
            # only on each bank's first matmul in program order (zero-region
            # semantics); everything else relies on pending-zero overwrite /
            # accumulate-on-written-bytes.
            psAB = pmm.tile([128, 2, 2, NR], F32)
            for b in range(BL):
                for fc in range(2):
                    nc.tensor.matmul(
                        psAB[:, fc, b, ds(0, S)],
                        lhsT=w1ab_s[:, ts(fc, 128)],
                        rhs=yb[:, b, :],
                        start=(b == 0 and fc == 0), stop=False,
                        skip_group_check=True,
                    )
                    nc.tensor.matmul(
                        psAB[:, fc, b, ds(S, S // 2)],
                        lhsT=w1a_s[:, ts(fc, 128)],
                        rhs=y4[:, 0 + b, :],
                        start=(b == 0 and fc == 1), stop=False,
                        skip_group_check=True,
                    )
                    nc.tensor.matmul(
                        psAB[:, fc, b, ds(S, S // 2)],
                        lhsT=w1b_s[:, ts(fc, 128)],
                        rhs=y4[:, 2 + b, :],
                        start=False, stop=(b == 1),
                        skip_group_check=True,
                    )
            h1 = singles.tile([128, 2, 2 * NR], F32)
            nc.scalar.activation(
                h1.rearrange("p a c -> p (a c)"),
                psAB.rearrange("p a b c -> p (a b c)"),
                AF.Relu,
            )

            # mc stride padded to 512 so each matmul output stays in one bank
            ps2 = pmm.tile([128, 2, 512], F32)
            for mc in range(2):
                for kc in range(2):
                    nc.tensor.matmul(
                        ps2[:, mc, ds(0, 2 * NR)],
                        lhsT=w2_s[:, kc, mc, :],
                        rhs=h1[:, kc, :],
                        start=(kc == 0),
                        stop=(kc == 1),
                    )
            h2 = singles.tile([128, 2, 2 * NR], F32)
            for mc in range(2):
                nc.scalar.activation(
                    h2[:, mc, :], ps2[:, mc, ds(0, 2 * NR)], AF.Relu,
                    bias=b2v_s[:, ds(mc, 1)],
                )

            ps3 = pmm.tile([10, 2 * NR], F32)
            for kc in range(2):
                nc.tensor.matmul(
                    ps3[:], lhsT=w3_s[:, kc, :], rhs=h2[:, kc, :],
                    start=(kc == 0), stop=(kc == 1),
                )
            h3 = singles.tile([10, 2 * NR], F32)
            nc.scalar.activation(h3[:], ps3[:], AF.Relu, bias=b3c_s[:, ds(0, 1)])

            ps4 = pmm.tile([2, 2 * NR], F32)  # logits [f, (b, row)]
            nc.tensor.matmul(ps4[:], lhsT=wt_s[:], rhs=h3[:], start=True, stop=True)

            # weighted lse over dim 0: log(64*sum exp lgA + 128*sum exp lgB)
            sA = singles.tile([2, BL], F32)
            sB = singles.tile([2, BL], F32)
            scr = singles.tile([2, 2 * NR], F32)
            for b in range(BL):
                nc.scalar.activation(
                    scr[:, ds(b * NR, S)], ps4[:, ds(b * NR, S)], AF.Exp,
                    accum_out=sA[:, ds(b, 1)],
                )
                nc.scalar.activation(
                    scr[:, ds(b * NR + S, S // 2)], ps4[:, ds(b * NR + S, S // 2)],
                    AF.Exp,
                    accum_out=sB[:, ds(b, 1)],
                )
            # B rows are counted 128x vs A's 64x: s = sA + 2*sB
            ssum = singles.tile([2, BL], F32)
            nc.vector.scalar_tensor_tensor(
                ssum[:], sB[:], 2.0, sA[:], op0=ALU.mult, op1=ALU.add
            )
            lse = singles.tile([2, BL], F32)
            nc.scalar.activation(lse[:], ssum[:], AF.Ln, scale=64.0)
            nlse = singles.tile([2, BL], F32)
            nc.vector.tensor_scalar_mul(nlse[:], lse[:], -1.0)

            lgAT = singles.tile([128, 2 * BL], F32)  # [i, (b, f)]
            # rowB[0, jj, b, f]: all B-region logits gathered on partition 0
            rowB = singles.tile([1, S // 2, BL, 2], F32)
            for b in range(BL):
                lg = work.tile([2, NR], F32, tag="lg")
                nc.scalar.activation(
                    lg[:], ps4[:, ds(b * NR, NR)], AF.Identity, bias=nlse[:, ds(b, 1)]
                )
                pA = ptr.tile([128, 2], F32, tag="pA")
                nc.tensor.transpose(pA[:], lg[:, ds(0, S)], eye2_s[:])
                nc.vector.tensor_copy(lgAT[:, ds(2 * b, 2)], pA[:])
                # gather the 2x64 B slice into the row (partition-crossing
                # DMAs, one per (b, f), spread over two queues)
                for fo in range(2):
                    eng = nc.sync if fo == 0 else nc.scalar
                    eng.dma_start(
                        out=rowB[:, :, b, fo],
                        in_=lg[ds(fo, 1), ds(S, S // 2)],
                    )

            # broadcast rowB over all 128 partitions via a K=1 ones matmul,
            # so the B-region DMA is a plain contiguous 1KB-per-partition copy
            psB = ptr.tile([128, S // 2 * BL * 2], F32, tag="psB")
            nc.tensor.matmul(
                psB[:],
                lhsT=ones_r[:],
                rhs=rowB.rearrange("p j b f -> p (j b f)"),
                start=True,
                stop=True,
            )
            sbB = singles.tile([128, S // 2 * BL * 2], F32)
            nc.vector.tensor_copy(sbB[:], psB[:])

            # region A (j < 64): value = lgAT[i, (b,f)] broadcast along j
            nc.sync.dma_start(
                out=out_d[:, 0:64, :], in_=bcast_free(lgAT[:, :], 64, 0)
            )
            # region B (j >= 64): contiguous per-partition copy
            nc.sync.dma_start(out=out_d[:, 64:128, :], in_=sbB[:])

        import os
        if os.environ.get("KERNEL_DEBUG_Y"):
            ydbg = nc.dram_tensor(
                "ydbg", [H + 1, 2 * (S + 1)], F32, kind="ExternalOutput"
            ).ap()
            nc.sync.dma_start(out=ydbg, in_=Y[:, :])


def build_nc():
    nc = bacc.Bacc(
        "TRN2",
        target_bir_lowering=False,
        debug=False,
        enable_asserts=False,
        num_devices=NCORES,
    )
    with tile.TileContext(nc) as tc:
        _emit(nc, tc)
    nc.compile()
    return nc


def prep_weights(W_ih, W_hh, b_ih, b_hh, W1, b1, W2, b2, W3, b3, Wt, bt):
    """Host-side weight preprocessing shared by all cores."""
    f = np.float32
    W_ih, W_hh = f(W_ih), f(W_hh)
    b_ih, b_hh = f(b_ih), f(b_hh)
    W1, b1, W2, b2 = f(W1), f(b1), f(W2), f(b2)
    W3, b3, Wt = f(W3), f(b3), f(Wt)

    def gate(W, bvec, g, sign=1.0):
        blk = np.concatenate(
            [W[g * H : (g + 1) * H].T, bvec[g * H : (g + 1) * H][None, :]], axis=0
        )
        return sign * blk

    # gate blocks [r, z'(= -z), n]: z' weights negated so sigmoid gives 1-z
    whh = np.concatenate(
        [gate(W_hh, b_hh, 0), gate(W_hh, b_hh, 1, -1.0), gate(W_hh, b_hh, 2)],
        axis=1,
    )
    wih = np.concatenate(
        [gate(W_ih, b_ih, 0), gate(W_ih, b_ih, 1, -1.0), gate(W_ih, b_ih, 2)],
        axis=1,
    )
    W1a, W1b = W1[:, :H], W1[:, H:]
    zrow = np.zeros((1, HID), np.float32)
    w1ab = np.concatenate([(W1a + W1b).T, b1[None, :]], axis=0)
    w1a_p = np.concatenate([W1a.T, b1[None, :]], axis=0)
    w1b_p = np.concatenate([W1b.T, zrow], axis=0)
    w2 = np.ascontiguousarray(W2.reshape(2, 128, 2, 128).transpose(3, 2, 0, 1))
    b2v = np.ascontiguousarray(b2.reshape(2, 128).T)
    w3 = np.ascontiguousarray(
        W3.reshape(10, 2, 128).transpose(2, 1, 0)
    )  # [p, kc, m]
    return {
        "whh": np.ascontiguousarray(whh),
        "wih": np.ascontiguousarray(wih),
        "w1ab": np.ascontiguousarray(w1ab),
        "w1a": np.ascontiguousarray(w1a_p),
        "w1b": np.ascontiguousarray(w1b_p),
        "w2": w2,
        "b2v": b2v,
        "w3": w3,
        "b3c": np.ascontiguousarray(b3[:, None]),
        "wt": np.ascontiguousarray(Wt.T),
        "eye2": np.eye(2, dtype=np.float32),
        "onesrow": np.ones((1, 128), np.float32),
    }


def make_in_maps(x, hidden, weights):
    x = np.asarray(x, np.float32)
    hidden = np.asarray(hidden, np.float32)
    in_maps = []
    for c in range(NCORES):
        b0 = c * BL
        xs = x[:, b0 : b0 + BL, :]
        xtc = np.concatenate(
            [xs.transpose(2, 0, 1).reshape(IN, S * BL),
             np.ones((1, S * BL), np.float32)], axis=0
        )
        m = dict(weights)
        m["xt"] = np.ascontiguousarray(xtc)
        y0 = np.ones((H + 1, 2 * (S + 1)), np.float32)
        y0[0:H, 0:BL] = hidden[0, b0 : b0 + BL, :].T
        m["y0"] = y0
        in_maps.append(m)
    return in_maps


def postprocess(results):
    outs = []
    for r in results:
        a = r["out"].reshape(S * S, BL, 2)
        outs.append(np.ascontiguousarray(a))
    return np.concatenate(outs, axis=1)


_NC_CACHE = {}


def get_nc():
    if "nc" not in _NC_CACHE:
        _NC_CACHE["nc"] = build_nc()
    return _NC_CACHE["nc"]


LAST_RESULTS = None


def kernel(x, hidden, W_ih, W_hh, b_ih, b_hh, W1, b1, W2, b2, W3, b3, Wt, bt,
           _run_kwargs=None):
    global LAST_RESULTS
    weights = prep_weights(W_ih, W_hh, b_ih, b_hh, W1, b1, W2, b2, W3, b3, Wt, bt)
    in_maps = make_in_maps(x, hidden, weights)
    nc = get_nc()
    res = run_bass_kernel_spmd(
        nc, in_maps, core_ids=list(range(NCORES)), **(_run_kwargs or {})
    )
    LAST_RESULTS = res
    return postprocess(res.results)


# revision 39
# speedup vs baseline: 1.5028x; 1.1848x over previous
"""Trainium2 Bass kernel for nn_Net_66451734004145 (GRU -> "adjacency" ->
MLP -> log_softmax over the S*S pair dim).

Key structural fact: the reference's adjacency reshape (faithful torch
translation) scrambles the pairwise concat.  For p = i*S + j:
    j <  S/2 : row = [y_i, y_i]            (depends only on i)
    j >= S/2 : row = [y_{2j-S}, y_{2j-S+1}] (depends only on j)
So the MLP has only S + S/2 = 192 distinct rows per batch element: 128
"A" rows (one per i) and 64 "B" rows (one per j-64).  The dim-0
log_softmax over all S*S rows reduces to
    lse = log(64*sum_i exp(lgA_i) + 128*sum_j exp(lgB_j))
and bt cancels (constant along dim 0).  The kernel computes the GRU (the
dominant, latency-bound part: 128 sequential steps), the 192-row MLP, the
weighted lse, and expands the output via broadcast DMAs.

Sharding: data-parallel over batch B=16 across 8 cores (2 per core); the
log_softmax dim stays local, no collectives.

GRU cell (feature-major [100, 2] state, biases folded via aug ones-row,
4th negated z-gate so 1-z comes from a sigmoid):
    psum_g = gi_g + gh_g accumulated by PE (g in r, z, z')
    r,z,z' = sigmoid(psum)        (one ACT op)
    n      = tanh(ghn * r + gin)  (ACT scale/bias [P,1] fusion, per b)
    g      = z * h                (DVE, per b)
    h'     = n * z' + g           (ACT Identity scale/bias, per b)

Output NEFF layout per core: [128, 128, 4] f32 = [i, j, (b,f)]; host
reshapes to (S*S, 2, 2) and concatenates over cores along batch.
"""

import contextlib
import math

import numpy as np

import concourse.bass as bass
import concourse.mybir as mybir
import concourse.tile as tile
from concourse import bacc
from concourse.bass import ds, ts
from concourse.bass_utils import run_bass_kernel_spmd

S = 128
B = 16
IN = 64
H = 100
HID = 256
NCORES = 8
BL = B // NCORES  # 2
NR = S + S // 2  # 192 distinct MLP rows per batch element

F32 = mybir.dt.float32
F32R = mybir.dt.float32r
AF = mybir.ActivationFunctionType
ALU = mybir.AluOpType

# blob packing: name -> (rows, cols); column offsets are cumulative
_BLOB_R_LAYOUT = [
    ("whh", H + 1, 3 * H),
    ("w1ab", H + 1, HID),
    ("w1a", H + 1, HID),
    ("w1b", H + 1, HID),
]
_BLOB_F_LAYOUT = [
    ("xt", IN + 1, S * BL),
    ("wih", IN + 1, 3 * H),
    ("w2", 128, 512),
    ("b2v", 128, 2),
    ("w3", 128, 20),
    ("b3c", 10, 1),
    ("wt", 10, 2),
    ("eye2", 2, 2),
    ("onesrow", 1, 128),
    ("ones64", 128, 64),
]


def _offsets(layout):
    off, o = {}, 0
    for name, _r, c in layout:
        off[name] = o
        o += c
    return off, o


BLOB_R_OFF, CR_BLOB = _offsets(_BLOB_R_LAYOUT)
BLOB_F_OFF, CF_BLOB = _offsets(_BLOB_F_LAYOUT)


def bcast_free(ap, n, axis):
    """Insert a broadcast (step 0, count n) free dim at free-axis position."""
    newap = [list(d) for d in ap.ap]
    newap.insert(1 + axis, [0, n])
    return bass.AP(tensor=ap.tensor, offset=ap.offset, ap=newap)


def _emit(nc, tc):
    # ---------------- DRAM I/O ----------------
    # All constant inputs are packed into two [128, C] blobs (one per dtype)
    # so input loading is 2 large contiguous DMAs instead of 14 small
    # descriptor-bound ones.  Individual weights are AP slices of the blobs.
    blob_r = nc.dram_tensor("blob_r", [128, CR_BLOB], F32R, kind="ExternalInput").ap()
    blob_f = nc.dram_tensor("blob_f", [128, CF_BLOB], F32, kind="ExternalInput").ap()
    y0 = nc.dram_tensor("y0", [H + 1, 2 * (S + 1)], F32R, kind="ExternalInput").ap()
    out_d = nc.dram_tensor("out", [S, S, 2 * BL], F32, kind="ExternalOutput").ap()

    with contextlib.ExitStack() as ctx:
        consts = ctx.enter_context(tc.tile_pool(name="consts", bufs=1))
        singles = ctx.enter_context(tc.tile_pool(name="singles", bufs=1))

        # activation-table warmup: tiny ops for every func used, so the
        # ACT_TABLE_LOADs run during the input DMAs instead of stalling the
        # first real use mid-kernel.
        wu = singles.tile([1, 4], F32)
        nc.vector.memset(wu[:, :], 1.0)
        for fn in (AF.Copy, AF.Sigmoid, AF.Tanh, AF.Identity, AF.Relu, AF.Exp,
                   AF.Ln):
            nc.scalar.activation(wu[:, 0:1], wu[:, 1:2], fn)

        blr = consts.tile([128, CR_BLOB], F32R, tag="blob_r")
        nc.gpsimd.dma_start(out=blr[:], in_=blob_r)
        blf = consts.tile([128, CF_BLOB], F32, tag="blob_f")
        nc.scalar.dma_start(out=blf[:], in_=blob_f)

        def off_r(name, rows, cols):
            return blr[0:rows, ds(BLOB_R_OFF[name], cols)]

        def off_f(name, rows, cols):
            return blf[0:rows, ds(BLOB_F_OFF[name], cols)]

        whh_s = off_r("whh", H + 1, 3 * H)
        w1ab_s = off_r("w1ab", H + 1, HID)
        w1a_s = off_r("w1a", H + 1, HID)
        w1b_s = off_r("w1b", H + 1, HID)
        xt_s = off_f("xt", IN + 1, S * BL)
        wih_s = off_f("wih", IN + 1, 3 * H)
        w2_s = off_f("w2", 128, 512).rearrange("p (a b c) -> p a b c", a=2, b=2)
        b2v_s = off_f("b2v", 128, 2)
        w3_s = off_f("w3", 128, 20).rearrange("p (a c) -> p a c", a=2)
        b3c_s = off_f("b3c", 10, 1)
        wt_s = off_f("wt", 10, 2)
        eye2_s = off_f("eye2", 2, 2)
        ones_r = off_f("onesrow", 1, 128)
        ones64_s = off_f("ones64", 128, 64)

        # Y holds [h_{-1}, h_0, ..., h_{127}] feature-major with an aug ones
        # row: Y[:, 2*(t+1)+b] = h_t for batch b.  memset everything to 1.0
        # (keeps the aug row); h0 DMA then covers rows 0:100, cols 0:2.
        # Y is float32r: the recurrent matmuls and the MLP's A/B matmuls
        # consume it on the PE; non-PE readers bitcast to f32.  Initial
        # content (aug ones row + h0 in cols 0:2) comes in via one DMA.
        Y = singles.tile([H + 1, 2 * (S + 1)], F32R)
        nc.sync.dma_start(out=Y[:, :], in_=y0)
        GIN = singles.tile([H, S * BL], F32)

        # ---------------- GRU ----------------
        with contextlib.ExitStack() as gru_ctx:
            pgi = gru_ctx.enter_context(tc.tile_pool(name="pgi", bufs=1, space="PSUM"))
            pghn = gru_ctx.enter_context(
                tc.tile_pool(name="pghn", bufs=2, space="PSUM")
            )
            rings = gru_ctx.enter_context(tc.tile_pool(name="rings", bufs=3))

            # PSUM start=True lazily zeroes a whole 2KB bank (zero region):
            # only the first matmul touching each bank may use start=True.
            # Layout [100, 3, 256]: gates r,z' (bank0), gin (bank1); each
            # gate block is first written by its GI matmul (start on bank
            # first-toucher only), then the per-step gh matmuls accumulate
            # into already-written bytes.
            # Cell: h' = z'*(n - h) + h with z' = sigmoid(-(i_z + h_z))
            # (z-gate weights negated on host), so no z gate is computed.
            psum_gi = pgi.tile([H, 3, S * BL], F32)

            for g in range(3):
                nc.tensor.matmul(
                    psum_gi[:, g, :],
                    lhsT=wih_s[:, ts(g, H)],
                    rhs=xt_s[:],
                    start=(g % 2 == 0),
                    stop=False,
                    skip_group_check=True,
                )
            nc.scalar.activation(GIN[:], psum_gi[:, 2, :], AF.Copy)

            for t in range(S):
                hcols = Y[:, ds(2 * t, 2)]
                for g in range(2):
                    nc.tensor.matmul(
                        psum_gi[:, g, ds(2 * t, 2)],
                        lhsT=whh_s[:, ts(g, H)],
                        rhs=hcols,
                        start=False,
                        stop=True,
                        skip_group_check=True,
                    )
                ghn = pghn.tile([H, BL], F32, tag="ghn")
                nc.tensor.matmul(
                    ghn[:], lhsT=whh_s[:, ts(2, H)], rhs=hcols,
                    start=True, stop=True,
                )
                rzp = rings.tile([H, 2, BL], F32, tag="rzp")
                nc.scalar.activation(
                    rzp[:], psum_gi[:, 0:2, ds(2 * t, 2)], AF.Sigmoid
                )
                ng = rings.tile([H, BL], F32, tag="ng")
                ee = rings.tile([H, BL], F32, tag="ee")
                for b in range(BL):
                    nc.scalar.activation(
                        ng[:, ds(b, 1)], ghn[:, ds(b, 1)], AF.Tanh,
                        scale=rzp[:, 0, ds(b, 1)],
                        bias=GIN[:, ds(2 * t + b, 1)],
                    )
                # h' = z'*(n - h) + h, all [100, 2] merged-batch DVE ops;
                # the final add writes f32r for the next step's matmul.
                mm_ = rings.tile([H, BL], F32, tag="mm")
                nc.vector.tensor_sub(
                    ee[:], ng[:], Y[0:H, ds(2 * t, 2)].bitcast(F32)
                )
                nc.vector.tensor_mul(mm_[:], ee[:], rzp[:, 1, :])
                nc.vector.tensor_add(
                    Y[0:H, ds(2 * (t + 1), 2)],
                    mm_[:],
                    Y[0:H, ds(2 * t, 2)].bitcast(F32),
                )

        # ---------------- 192-row MLP + lse + output expansion ------------
        # column views of Y: all y_t for batch b / even t / odd t
        yb = Y[:, ds(2, 2 * S)].rearrange("p (i bb) -> p bb i", bb=2)
        y4 = Y[:, ds(2, 2 * S)].rearrange("p (k f) -> p f k", f=4)
        # y4[:, 2k + b, :] == y_{2j+k} columns for batch b

        with contextlib.ExitStack() as mlp_ctx:
            pmm = mlp_ctx.enter_context(tc.tile_pool(name="pmm", bufs=1, space="PSUM"))
            ptr = mlp_ctx.enter_context(tc.tile_pool(name="ptr", bufs=1, space="PSUM"))
            work = mlp_ctx.enter_context(tc.tile_pool(name="work", bufs=2))

            # [p, fc, b, row]; bank0 = cols 0:512, bank1 = 512:768.  start=True
            # only on each bank's first matmul in program order (zero-region
            # semantics); everything else relies on pending-zero overwrite /
            # accumulate-on-written-bytes.
            psAB = pmm.tile([128, 2, 2, NR], F32)
            for b in range(BL):
                for fc in range(2):
                    nc.tensor.matmul(
                        psAB[:, fc, b, ds(0, S)],
                        lhsT=w1ab_s[:, ts(fc, 128)],
                        rhs=yb[:, b, :],
                        start=(b == 0 and fc == 0), stop=False,
                        skip_group_check=True,
                    )
                    nc.tensor.matmul(
                        psAB[:, fc, b, ds(S, S // 2)],
                        lhsT=w1a_s[:, ts(fc, 128)],
                        rhs=y4[:, 0 + b, :],
                        start=(b == 0 and fc == 1), stop=False,
                        skip_group_check=True,
                    )
                    nc.tensor.matmul(
                        psAB[:, fc, b, ds(S, S // 2)],
                        lhsT=w1b_s[:, ts(fc, 128)],
                        rhs=y4[:, 2 + b, :],
                        start=False, stop=(b == 1),
                        skip_group_check=True,
                    )
            h1 = singles.tile([128, 2, 2 * NR], F32)
            nc.scalar.activation(
                h1.rearrange("p a c -> p (a c)"),
                psAB.rearrange("p a b c -> p (a b c)"),
                AF.Relu,
            )

            # mc stride padded to 512 so each matmul output stays in one bank
            ps2 = pmm.tile([128, 2, 512], F32)
            for mc in range(2):
                for kc in range(2):
                    nc.tensor.matmul(
                        ps2[:, mc, ds(0, 2 * NR)],
                        lhsT=w2_s[:, kc, mc, :],
                        rhs=h1[:, kc, :],
                        start=(kc == 0),
                        stop=(kc == 1),
                    )
            h2 = singles.tile([128, 2, 2 * NR], F32)
            for mc in range(2):
                nc.scalar.activation(
                    h2[:, mc, :], ps2[:, mc, ds(0, 2 * NR)], AF.Relu,
                    bias=b2v_s[:, ds(mc, 1)],
                )

            ps3 = pmm.tile([10, 2 * NR], F32)
            for kc in range(2):
                nc.tensor.matmul(
                    ps3[:], lhsT=w3_s[:, kc, :], rhs=h2[:, kc, :],
                    start=(kc == 0), stop=(kc == 1),
                )
            h3 = singles.tile([10, 2 * NR], F32)
            nc.scalar.activation(h3[:], ps3[:], AF.Relu, bias=b3c_s[:, ds(0, 1)])

            ps4 = pmm.tile([2, 2 * NR], F32)  # logits [f, (b, row)]
            nc.tensor.matmul(ps4[:], lhsT=wt_s[:], rhs=h3[:], start=True, stop=True)

            # weighted lse over dim 0: log(64*sum exp lgA + 128*sum exp lgB)
            sA = singles.tile([2, BL], F32)
            sB = singles.tile([2, BL], F32)
            scr = singles.tile([2, 2 * NR], F32)
            for b in range(BL):
                nc.scalar.activation(
                    scr[:, ds(b * NR, S)], ps4[:, ds(b * NR, S)], AF.Exp,
                    accum_out=sA[:, ds(b, 1)],
                )
                nc.scalar.activation(
                    scr[:, ds(b * NR + S, S // 2)], ps4[:, ds(b * NR + S, S // 2)],
                    AF.Exp,
                    accum_out=sB[:, ds(b, 1)],
                )
            # B rows are counted 128x vs A's 64x: s = sA + 2*sB
            ssum = singles.tile([2, BL], F32)
            nc.vector.scalar_tensor_tensor(
                ssum[:], sB[:], 2.0, sA[:], op0=ALU.mult, op1=ALU.add
            )
            lse = singles.tile([2, BL], F32)
            nc.scalar.activation(lse[:], ssum[:], AF.Ln, scale=64.0)
            nlse = singles.tile([2, BL], F32)
            nc.vector.tensor_scalar_mul(nlse[:], lse[:], -1.0)

            lgAT = singles.tile([128, 2 * BL], F32)  # [i, (b, f)]
            # rowB[0, jj, b, f]: all B-region logits gathered on partition 0
            rowB = singles.tile([1, S // 2, BL, 2], F32)
            for b in range(BL):
                lg = work.tile([2, NR], F32, tag="lg")
                nc.scalar.activation(
                    lg[:], ps4[:, ds(b * NR, NR)], AF.Identity, bias=nlse[:, ds(b, 1)]
                )
                pA = ptr.tile([128, 2], F32, tag="pA")
                nc.tensor.transpose(pA[:], lg[:, ds(0, S)], eye2_s[:])
                nc.vector.tensor_copy(lgAT[:, ds(2 * b, 2)], pA[:])
                # gather the 2x64 B slice into the row (partition-crossing
                # DMAs, one per (b, f), spread over two queues)
                for fo in range(2):
                    eng = nc.sync if fo == 0 else nc.scalar
                    eng.dma_start(
                        out=rowB[:, :, b, fo],
                        in_=lg[ds(fo, 1), ds(S, S // 2)],
                    )

            # broadcast rowB over all 128 partitions via a K=1 ones matmul,
            # so the B-region DMA is a plain contiguous 1KB-per-partition copy
            psB = ptr.tile([128, S // 2 * BL * 2], F32, tag="psB")
            nc.tensor.matmul(
                psB[:],
                lhsT=ones_r[:],
                rhs=rowB.rearrange("p j b f -> p (j b f)"),
                start=True,
                stop=True,
            )
            sbB = singles.tile([128, S // 2 * BL * 2], F32)
            nc.vector.tensor_copy(sbB[:], psB[:])

            # region A (j < 64): value = lgAT[i, (b,f)] broadcast along j,
            # materialized by DVE (ones * per-partition scalar) so the DMA
            # is a plain contiguous copy (broadcast-read DMAs are ~40x
            # slower).
            sbA = singles.tile([128, 64, BL, 2], F32)
            for b in range(BL):
                for fo in range(2):
                    nc.vector.tensor_scalar_mul(
                        sbA[:, :, b, fo], ones64_s, lgAT[:, ds(2 * b + fo, 1)]
                    )
            nc.sync.dma_start(
                out=out_d[:, 0:64, :], in_=sbA.rearrange("p j b f -> p (j b f)")
            )
            # region B (j >= 64): contiguous per-partition copy
            nc.scalar.dma_start(out=out_d[:, 64:128, :], in_=sbB[:])

        import os
        if os.environ.get("KERNEL_DEBUG_Y"):
            ydbg = nc.dram_tensor(
                "ydbg", [H + 1, 2 * (S + 1)], F32, kind="ExternalOutput"
            ).ap()
            nc.sync.dma_start(out=ydbg, in_=Y[:, :])


def build_nc():
    nc = bacc.Bacc(
        "TRN2",
        target_bir_lowering=False,
        debug=False,
        enable_asserts=False,
        num_devices=NCORES,
    )
    with tile.TileContext(nc) as tc:
        _emit(nc, tc)
    nc.compile()
    return nc


def prep_weights(W_ih, W_hh, b_ih, b_hh, W1, b1, W2, b2, W3, b3, Wt, bt):
    """Host-side weight preprocessing shared by all cores: builds the two
    constant input blobs (f32r and f32)."""
    f = np.float32
    W_ih, W_hh = f(W_ih), f(W_hh)
    b_ih, b_hh = f(b_ih), f(b_hh)
    W1, b1, W2, b2 = f(W1), f(b1), f(W2), f(b2)
    W3, b3, Wt = f(W3), f(b3), f(Wt)

    def gate(W, bvec, g, sign=1.0):
        blk = np.concatenate(
            [W[g * H : (g + 1) * H].T, bvec[g * H : (g + 1) * H][None, :]], axis=0
        )
        return sign * blk

    # gate blocks [r, z'(= -z), n]: z' weights negated so sigmoid gives 1-z
    whh = np.concatenate(
        [gate(W_hh, b_hh, 0), gate(W_hh, b_hh, 1, -1.0), gate(W_hh, b_hh, 2)],
        axis=1,
    )
    wih = np.concatenate(
        [gate(W_ih, b_ih, 0), gate(W_ih, b_ih, 1, -1.0), gate(W_ih, b_ih, 2)],
        axis=1,
    )
    W1a, W1b = W1[:, :H], W1[:, H:]
    zrow = np.zeros((1, HID), np.float32)
    parts = {
        "whh": whh,
        "w1ab": np.concatenate([(W1a + W1b).T, b1[None, :]], axis=0),
        "w1a": np.concatenate([W1a.T, b1[None, :]], axis=0),
        "w1b": np.concatenate([W1b.T, zrow], axis=0),
        "wih": wih,
        "w2": W2.reshape(2, 128, 2, 128).transpose(3, 2, 0, 1).reshape(128, 512),
        "b2v": b2.reshape(2, 128).T,
        "w3": W3.reshape(10, 2, 128).transpose(2, 1, 0).reshape(128, 20),
        "b3c": b3[:, None],
        "wt": Wt.T,
        "eye2": np.eye(2, dtype=np.float32),
        "onesrow": np.ones((1, 128), np.float32),
        "ones64": np.ones((128, 64), np.float32),
    }

    def build(layout, offs, width):
        blob = np.zeros((128, width), np.float32)
        for name, rows, cols in layout:
            a = np.asarray(parts[name], np.float32)
            assert a.shape == (rows, cols), (name, a.shape, rows, cols)
            blob[0:rows, offs[name] : offs[name] + cols] = a
        return blob

    return {
        "blob_r": build(_BLOB_R_LAYOUT, BLOB_R_OFF, CR_BLOB),
        "blob_f": None,  # needs xt per core; see make_in_maps
        "_parts": parts,
    }


def make_in_maps(x, hidden, weights):
    x = np.asarray(x, np.float32)
    hidden = np.asarray(hidden, np.float32)
    parts = weights["_parts"]
    in_maps = []
    for c in range(NCORES):
        b0 = c * BL
        xs = x[:, b0 : b0 + BL, :]
        xtc = np.concatenate(
            [xs.transpose(2, 0, 1).reshape(IN, S * BL),
             np.ones((1, S * BL), np.float32)], axis=0
        )
        p = dict(parts)
        p["xt"] = xtc
        blob_f = np.zeros((128, CF_BLOB), np.float32)
        for name, rows, cols in _BLOB_F_LAYOUT:
            a = np.asarray(p[name], np.float32)
            assert a.shape == (rows, cols), (name, a.shape)
            blob_f[0:rows, BLOB_F_OFF[name] : BLOB_F_OFF[name] + cols] = a
        y0 = np.ones((H + 1, 2 * (S + 1)), np.float32)
        y0[0:H, 0:BL] = hidden[0, b0 : b0 + BL, :].T
        in_maps.append({
            "blob_r": weights["blob_r"],
            "blob_f": blob_f,
            "y0": y0,
        })
    return in_maps


def postprocess(results):
    outs = []
    for r in results:
        a = r["out"].reshape(S * S, BL, 2)
        outs.append(np.ascontiguousarray(a))
    return np.concatenate(outs, axis=1)


_NC_CACHE = {}


def get_nc():
    if "nc" not in _NC_CACHE:
        _NC_CACHE["nc"] = build_nc()
    return _NC_CACHE["nc"]


LAST_RESULTS = None


def kernel(x, hidden, W_ih, W_hh, b_ih, b_hh, W1, b1, W2, b2, W3, b3, Wt, bt,
           _run_kwargs=None):
    global LAST_RESULTS
    weights = prep_weights(W_ih, W_hh, b_ih, b_hh, W1, b1, W2, b2, W3, b3, Wt, bt)
    in_maps = make_in_maps(x, hidden, weights)
    nc = get_nc()
    res = run_bass_kernel_spmd(
        nc, in_maps, core_ids=list(range(NCORES)), **(_run_kwargs or {})
    )
    LAST_RESULTS = res
    return postprocess(res.results)


# revision 44
# speedup vs baseline: 1.5444x; 1.0277x over previous
"""Trainium2 Bass kernel for nn_Net_66451734004145 (GRU -> "adjacency" ->
MLP -> log_softmax over the S*S pair dim).

Key structural fact: the reference's adjacency reshape (faithful torch
translation) scrambles the pairwise concat.  For p = i*S + j:
    j <  S/2 : row = [y_i, y_i]            (depends only on i)
    j >= S/2 : row = [y_{2j-S}, y_{2j-S+1}] (depends only on j)
So the MLP has only S + S/2 = 192 distinct rows per batch element: 128
"A" rows (one per i) and 64 "B" rows (one per j-64).  The dim-0
log_softmax over all S*S rows reduces to
    lse = log(64*sum_i exp(lgA_i) + 128*sum_j exp(lgB_j))
and bt cancels (constant along dim 0).  The kernel computes the GRU (the
dominant, latency-bound part: 128 sequential steps), the 192-row MLP, the
weighted lse, and expands the output via broadcast DMAs.

Sharding: data-parallel over batch B=16 across 8 cores (2 per core); the
log_softmax dim stays local, no collectives.

GRU cell (feature-major [100, 2] state, biases folded via aug ones-row,
4th negated z-gate so 1-z comes from a sigmoid):
    psum_g = gi_g + gh_g accumulated by PE (g in r, z, z')
    r,z,z' = sigmoid(psum)        (one ACT op)
    n      = tanh(ghn * r + gin)  (ACT scale/bias [P,1] fusion, per b)
    g      = z * h                (DVE, per b)
    h'     = n * z' + g           (ACT Identity scale/bias, per b)

Output NEFF layout per core: [128, 128, 4] f32 = [i, j, (b,f)]; host
reshapes to (S*S, 2, 2) and concatenates over cores along batch.
"""

import contextlib
import math

import numpy as np

import concourse.bass as bass
import concourse.mybir as mybir
import concourse.tile as tile
from concourse import bacc
from concourse.bass import ds, ts
from concourse.bass_utils import run_bass_kernel_spmd

S = 128
B = 16
IN = 64
H = 100
HID = 256
NCORES = 8
BL = B // NCORES  # 2
NR = S + S // 2  # 192 distinct MLP rows per batch element

F32 = mybir.dt.float32
F32R = mybir.dt.float32r
AF = mybir.ActivationFunctionType
ALU = mybir.AluOpType

# blob packing: name -> (rows, cols); column offsets are cumulative.
# hot blobs land first (GRU-critical), cold holds everything the MLP tail
# needs; split across DMA queues so completion isn't serialized.
_BLOB_HOT_LAYOUT = [          # f32r, sync queue (GRU weights)
    ("whh", H + 1, 3 * H),
    ("wih", IN + 1, 3 * H),
]
_BLOB_XT_LAYOUT = [           # f32r, gpsimd queue
    ("xt", IN + 1, S * BL),
]
_BLOB_COLD_LAYOUT = [         # f32r, gpsimd queue (MLP weights)
    ("w1ab", H + 1, HID),
    ("w1a", H + 1, HID),
    ("w1b", H + 1, HID),
    ("w2", 128, 512),
    ("w3", 128, 20),
    ("wt", 10, 2),
    ("eye2", 2, 2),
    ("onesrow", 1, 128),
]
_BLOB_F_LAYOUT = [            # f32, scalar queue (non-PE operands)
    ("b2v", 128, 2),
    ("b3c", 10, 1),
    ("ones64", 128, 64),
]


def _offsets(layout):
    off, o = {}, 0
    for name, _r, c in layout:
        off[name] = o
        o += c
    return off, o


BLOB_HOT_OFF, C_HOT = _offsets(_BLOB_HOT_LAYOUT)
BLOB_XT_OFF, C_XT = _offsets(_BLOB_XT_LAYOUT)
BLOB_COLD_OFF, C_COLD = _offsets(_BLOB_COLD_LAYOUT)
BLOB_F_OFF, C_F = _offsets(_BLOB_F_LAYOUT)


def bcast_free(ap, n, axis):
    """Insert a broadcast (step 0, count n) free dim at free-axis position."""
    newap = [list(d) for d in ap.ap]
    newap.insert(1 + axis, [0, n])
    return bass.AP(tensor=ap.tensor, offset=ap.offset, ap=newap)


def _emit(nc, tc):
    # ---------------- DRAM I/O ----------------
    bhot = nc.dram_tensor("bhot", [128, C_HOT], F32R, kind="ExternalInput").ap()
    bxt = nc.dram_tensor("bxt", [128, C_XT], F32R, kind="ExternalInput").ap()
    bcold = nc.dram_tensor("bcold", [128, C_COLD], F32R, kind="ExternalInput").ap()
    bf = nc.dram_tensor("bf", [128, C_F], F32, kind="ExternalInput").ap()
    y0 = nc.dram_tensor("y0", [H + 1, 2 * (S + 1)], F32R, kind="ExternalInput").ap()
    out_d = nc.dram_tensor("out", [S, S, 2 * BL], F32, kind="ExternalOutput").ap()

    with contextlib.ExitStack() as ctx:
        consts = ctx.enter_context(tc.tile_pool(name="consts", bufs=1))
        singles = ctx.enter_context(tc.tile_pool(name="singles", bufs=1))

        # activation-table warmup: tiny ops ordered so the LAST one leaves
        # the sigmoid/tanh table set resident for the GRU.
        wu = singles.tile([1, 4], F32)
        nc.vector.memset(wu[:, :], 1.0)
        for fn in (AF.Copy, AF.Exp, AF.Ln, AF.Sigmoid):
            nc.scalar.activation(wu[:, 0:1], wu[:, 1:2], fn)

        t_hot = consts.tile([128, C_HOT], F32R, tag="bhot")
        nc.sync.dma_start(out=t_hot[:], in_=bhot)
        t_xt = consts.tile([128, C_XT], F32R, tag="bxt")
        nc.gpsimd.dma_start(out=t_xt[:], in_=bxt)
        Y = singles.tile([H + 1, 2 * (S + 1)], F32R)
        nc.scalar.dma_start(out=Y[:, :], in_=y0)
        t_cold = consts.tile([128, C_COLD], F32R, tag="bcold")
        nc.gpsimd.dma_start(out=t_cold[:], in_=bcold)
        t_f = consts.tile([128, C_F], F32, tag="bf")
        nc.scalar.dma_start(out=t_f[:], in_=bf)

        def sl(tileap, offs, name, rows, cols):
            return tileap[0:rows, ds(offs[name], cols)]

        whh_s = sl(t_hot, BLOB_HOT_OFF, "whh", H + 1, 3 * H)
        wih_s = sl(t_hot, BLOB_HOT_OFF, "wih", IN + 1, 3 * H)
        xt_s = sl(t_xt, BLOB_XT_OFF, "xt", IN + 1, S * BL)
        w1ab_s = sl(t_cold, BLOB_COLD_OFF, "w1ab", H + 1, HID)
        w1a_s = sl(t_cold, BLOB_COLD_OFF, "w1a", H + 1, HID)
        w1b_s = sl(t_cold, BLOB_COLD_OFF, "w1b", H + 1, HID)
        w2_s = sl(t_cold, BLOB_COLD_OFF, "w2", 128, 512).rearrange(
            "p (a b c) -> p a b c", a=2, b=2
        )
        w3_s = sl(t_cold, BLOB_COLD_OFF, "w3", 128, 20).rearrange(
            "p (a c) -> p a c", a=2
        )
        wt_s = sl(t_cold, BLOB_COLD_OFF, "wt", 10, 2)
        eye2_s = sl(t_cold, BLOB_COLD_OFF, "eye2", 2, 2)
        ones_r = sl(t_cold, BLOB_COLD_OFF, "onesrow", 1, 128)
        b2v_s = sl(t_f, BLOB_F_OFF, "b2v", 128, 2)
        b3c_s = sl(t_f, BLOB_F_OFF, "b3c", 10, 1)
        ones64_s = sl(t_f, BLOB_F_OFF, "ones64", 128, 64)

        # Y holds [h_{-1}, h_0, ..., h_{127}] feature-major with an aug ones
        # row: Y[:, 2*(t+1)+b] = h_t for batch b (f32r; loaded above).
        GIN = singles.tile([H, S * BL], F32)

        # ---------------- GRU ----------------
        with contextlib.ExitStack() as gru_ctx:
            pgi = gru_ctx.enter_context(tc.tile_pool(name="pgi", bufs=1, space="PSUM"))
            pghn = gru_ctx.enter_context(
                tc.tile_pool(name="pghn", bufs=2, space="PSUM")
            )
            rings = gru_ctx.enter_context(tc.tile_pool(name="rings", bufs=3))

            # PSUM start=True lazily zeroes a whole 2KB bank (zero region):
            # only the first matmul touching each bank may use start=True.
            # Layout [100, 3, 256]: gates r,z' (bank0), gin (bank1); each
            # gate block is first written by its GI matmul (start on bank
            # first-toucher only), then the per-step gh matmuls accumulate
            # into already-written bytes.
            # Cell: h' = z'*(n - h) + h with z' = sigmoid(-(i_z + h_z))
            # (z-gate weights negated on host), so no z gate is computed.
            psum_gi = pgi.tile([H, 3, S * BL], F32)

            for g in range(3):
                nc.tensor.matmul(
                    psum_gi[:, g, :],
                    lhsT=wih_s[:, ts(g, H)],
                    rhs=xt_s[:],
                    start=(g % 2 == 0),
                    stop=False,
                    skip_group_check=True,
                )
            nc.scalar.activation(GIN[:], psum_gi[:, 2, :], AF.Copy)

            for t in range(S):
                hcols = Y[:, ds(2 * t, 2)]
                for g in range(2):
                    nc.tensor.matmul(
                        psum_gi[:, g, ds(2 * t, 2)],
                        lhsT=whh_s[:, ts(g, H)],
                        rhs=hcols,
                        start=False,
                        stop=True,
                        skip_group_check=True,
                    )
                ghn = pghn.tile([H, BL], F32, tag="ghn")
                nc.tensor.matmul(
                    ghn[:], lhsT=whh_s[:, ts(2, H)], rhs=hcols,
                    start=True, stop=True,
                )
                rzp = rings.tile([H, 2, BL], F32, tag="rzp")
                nc.scalar.activation(
                    rzp[:], psum_gi[:, 0:2, ds(2 * t, 2)], AF.Sigmoid
                )
                ng = rings.tile([H, BL], F32, tag="ng")
                ee = rings.tile([H, BL], F32, tag="ee")
                for b in range(BL):
                    nc.scalar.activation(
                        ng[:, ds(b, 1)], ghn[:, ds(b, 1)], AF.Tanh,
                        scale=rzp[:, 0, ds(b, 1)],
                        bias=GIN[:, ds(2 * t + b, 1)],
                    )
                # h' = z'*(n - h) + h, all [100, 2] merged-batch DVE ops;
                # the final add writes f32r for the next step's matmul.
                mm_ = rings.tile([H, BL], F32, tag="mm")
                nc.vector.tensor_sub(
                    ee[:], ng[:], Y[0:H, ds(2 * t, 2)].bitcast(F32)
                )
                nc.vector.tensor_mul(mm_[:], ee[:], rzp[:, 1, :])
                nc.vector.tensor_add(
                    Y[0:H, ds(2 * (t + 1), 2)],
                    mm_[:],
                    Y[0:H, ds(2 * t, 2)].bitcast(F32),
                )

        # ---------------- 192-row MLP + lse + output expansion ------------
        # column views of Y: all y_t for batch b / even t / odd t
        yb = Y[:, ds(2, 2 * S)].rearrange("p (i bb) -> p bb i", bb=2)
        y4 = Y[:, ds(2, 2 * S)].rearrange("p (k f) -> p f k", f=4)
        # y4[:, 2k + b, :] == y_{2j+k} columns for batch b

        with contextlib.ExitStack() as mlp_ctx:
            pmm = mlp_ctx.enter_context(tc.tile_pool(name="pmm", bufs=1, space="PSUM"))
            ptr = mlp_ctx.enter_context(tc.tile_pool(name="ptr", bufs=1, space="PSUM"))
            work = mlp_ctx.enter_context(tc.tile_pool(name="work", bufs=2))

            # [p, fc, b, row]; bank0 = cols 0:512, bank1 = 512:768.  start=True
            # only on each bank's first matmul in program order (zero-region
            # semantics); everything else relies on pending-zero overwrite /
            # accumulate-on-written-bytes.
            psAB = pmm.tile([128, 2, 2, NR], F32)
            for b in range(BL):
                for fc in range(2):
                    nc.tensor.matmul(
                        psAB[:, fc, b, ds(0, S)],
                        lhsT=w1ab_s[:, ts(fc, 128)],
                        rhs=yb[:, b, :],
                        start=(b == 0 and fc == 0), stop=False,
                        skip_group_check=True,
                    )
                    nc.tensor.matmul(
                        psAB[:, fc, b, ds(S, S // 2)],
                        lhsT=w1a_s[:, ts(fc, 128)],
                        rhs=y4[:, 0 + b, :],
                        start=(b == 0 and fc == 1), stop=False,
                        skip_group_check=True,
                    )
                    nc.tensor.matmul(
                        psAB[:, fc, b, ds(S, S // 2)],
                        lhsT=w1b_s[:, ts(fc, 128)],
                        rhs=y4[:, 2 + b, :],
                        start=False, stop=(b == 1),
                        skip_group_check=True,
                    )
            h1 = singles.tile([128, 2, 2 * NR], F32R)
            nc.vector.tensor_scalar_max(
                h1.rearrange("p a c -> p (a c)"),
                psAB.rearrange("p a b c -> p (a b c)"),
                0.0,
            )

            # mc stride padded to 512 so each matmul output stays in one bank
            ps2 = pmm.tile([128, 2, 512], F32)
            for mc in range(2):
                for kc in range(2):
                    nc.tensor.matmul(
                        ps2[:, mc, ds(0, 2 * NR)],
                        lhsT=w2_s[:, kc, mc, :],
                        rhs=h1[:, kc, :],
                        start=(kc == 0),
                        stop=(kc == 1),
                    )
            h2 = singles.tile([128, 2, 2 * NR], F32R)
            for mc in range(2):
                nc.vector.tensor_scalar(
                    h2[:, mc, :], ps2[:, mc, ds(0, 2 * NR)],
                    b2v_s[:, ds(mc, 1)], 0.0, op0=ALU.add, op1=ALU.max,
                )

            ps3 = pmm.tile([10, 2 * NR], F32)
            for kc in range(2):
                nc.tensor.matmul(
                    ps3[:], lhsT=w3_s[:, kc, :], rhs=h2[:, kc, :],
                    start=(kc == 0), stop=(kc == 1),
                )
            h3 = singles.tile([10, 2 * NR], F32R)
            nc.vector.tensor_scalar(
                h3[:], ps3[:], b3c_s[:, ds(0, 1)], 0.0, op0=ALU.add, op1=ALU.max
            )

            ps4 = pmm.tile([2, 2 * NR], F32)  # logits [f, (b, row)]
            nc.tensor.matmul(ps4[:], lhsT=wt_s[:], rhs=h3[:], start=True, stop=True)

            # weighted lse over dim 0: log(64*sum exp lgA + 128*sum exp lgB)
            sA = singles.tile([2, BL], F32)
            sB = singles.tile([2, BL], F32)
            scr = singles.tile([2, 2 * NR], F32)
            for b in range(BL):
                nc.scalar.activation(
                    scr[:, ds(b * NR, S)], ps4[:, ds(b * NR, S)], AF.Exp,
                    accum_out=sA[:, ds(b, 1)],
                )
                nc.scalar.activation(
                    scr[:, ds(b * NR + S, S // 2)], ps4[:, ds(b * NR + S, S // 2)],
                    AF.Exp,
                    accum_out=sB[:, ds(b, 1)],
                )
            # B rows are counted 128x vs A's 64x: s = sA + 2*sB
            ssum = singles.tile([2, BL], F32)
            nc.vector.scalar_tensor_tensor(
                ssum[:], sB[:], 2.0, sA[:], op0=ALU.mult, op1=ALU.add
            )
            lse = singles.tile([2, BL], F32)
            nc.scalar.activation(lse[:], ssum[:], AF.Ln, scale=64.0)
            nlse = singles.tile([2, BL], F32)
            nc.vector.tensor_scalar_mul(nlse[:], lse[:], -1.0)

            lgAT = singles.tile([128, 2 * BL], F32)  # [i, (b, f)]
            # rowB[0, jj, b, f]: all B-region logits gathered on partition 0
            rowB = singles.tile([1, S // 2, BL, 2], F32R)
            for b in range(BL):
                lg = work.tile([2, NR], F32R, tag="lg")
                nc.vector.tensor_scalar_add(
                    lg[:], ps4[:, ds(b * NR, NR)], nlse[:, ds(b, 1)]
                )
                pA = ptr.tile([128, 2], F32R, tag="pA")
                nc.tensor.transpose(pA[:], lg[:, ds(0, S)], eye2_s[:])
                nc.vector.tensor_copy(lgAT[:, ds(2 * b, 2)], pA[:].bitcast(F32))
                # gather the 2x64 B slice into the row (partition-crossing
                # DMAs, one per (b, f), spread over two queues)
                for fo in range(2):
                    eng = nc.sync if fo == 0 else nc.scalar
                    eng.dma_start(
                        out=rowB[:, :, b, fo],
                        in_=lg[ds(fo, 1), ds(S, S // 2)],
                    )

            # broadcast rowB over all 128 partitions via a K=1 ones matmul,
            # so the B-region DMA is a plain contiguous 1KB-per-partition copy
            psB = ptr.tile([128, S // 2 * BL * 2], F32, tag="psB")
            nc.tensor.matmul(
                psB[:],
                lhsT=ones_r[:],
                rhs=rowB.rearrange("p j b f -> p (j b f)"),
                start=True,
                stop=True,
            )
            sbB = singles.tile([128, S // 2 * BL * 2], F32)
            nc.vector.tensor_copy(sbB[:], psB[:])

            # region A (j < 64): value = lgAT[i, (b,f)] broadcast along j,
            # materialized by DVE (ones * per-partition scalar) so the DMA
            # is a plain contiguous copy (broadcast-read DMAs are ~40x
            # slower).
            sbA = singles.tile([128, 64, BL, 2], F32)
            for b in range(BL):
                for fo in range(2):
                    nc.vector.tensor_scalar_mul(
                        sbA[:, :, b, fo], ones64_s, lgAT[:, ds(2 * b + fo, 1)]
                    )
            nc.sync.dma_start(
                out=out_d[:, 0:64, :], in_=sbA.rearrange("p j b f -> p (j b f)")
            )
            # region B (j >= 64): contiguous per-partition copy
            nc.scalar.dma_start(out=out_d[:, 64:128, :], in_=sbB[:])

        import os
        if os.environ.get("KERNEL_DEBUG_Y"):
            ydbg = nc.dram_tensor(
                "ydbg", [H + 1, 2 * (S + 1)], F32, kind="ExternalOutput"
            ).ap()
            nc.sync.dma_start(out=ydbg, in_=Y[:, :])


def build_nc():
    nc = bacc.Bacc(
        "TRN2",
        target_bir_lowering=False,
        debug=False,
        enable_asserts=False,
        num_devices=NCORES,
    )
    with tile.TileContext(nc) as tc:
        _emit(nc, tc)
    nc.compile()
    return nc


def prep_weights(W_ih, W_hh, b_ih, b_hh, W1, b1, W2, b2, W3, b3, Wt, bt):
    """Host-side weight preprocessing shared by all cores."""
    f = np.float32
    W_ih, W_hh = f(W_ih), f(W_hh)
    b_ih, b_hh = f(b_ih), f(b_hh)
    W1, b1, W2, b2 = f(W1), f(b1), f(W2), f(b2)
    W3, b3, Wt = f(W3), f(b3), f(Wt)

    def gate(W, bvec, g, sign=1.0):
        blk = np.concatenate(
            [W[g * H : (g + 1) * H].T, bvec[g * H : (g + 1) * H][None, :]], axis=0
        )
        return sign * blk

    # gate blocks [r, z'(= -z), n]: z' weights negated so sigmoid gives 1-z
    whh = np.concatenate(
        [gate(W_hh, b_hh, 0), gate(W_hh, b_hh, 1, -1.0), gate(W_hh, b_hh, 2)],
        axis=1,
    )
    wih = np.concatenate(
        [gate(W_ih, b_ih, 0), gate(W_ih, b_ih, 1, -1.0), gate(W_ih, b_ih, 2)],
        axis=1,
    )
    W1a, W1b = W1[:, :H], W1[:, H:]
    zrow = np.zeros((1, HID), np.float32)
    parts = {
        "whh": whh,
        "wih": wih,
        "w1ab": np.concatenate([(W1a + W1b).T, b1[None, :]], axis=0),
        "w1a": np.concatenate([W1a.T, b1[None, :]], axis=0),
        "w1b": np.concatenate([W1b.T, zrow], axis=0),
        "w2": W2.reshape(2, 128, 2, 128).transpose(3, 2, 0, 1).reshape(128, 512),
        "b2v": b2.reshape(2, 128).T,
        "w3": W3.reshape(10, 2, 128).transpose(2, 1, 0).reshape(128, 20),
        "b3c": b3[:, None],
        "wt": Wt.T,
        "eye2": np.eye(2, dtype=np.float32),
        "onesrow": np.ones((1, 128), np.float32),
        "ones64": np.ones((128, 64), np.float32),
    }

    def build(layout, offs, width):
        blob = np.zeros((128, width), np.float32)
        for name, rows, cols in layout:
            a = np.asarray(parts[name], np.float32)
            assert a.shape == (rows, cols), (name, a.shape, rows, cols)
            blob[0:rows, offs[name] : offs[name] + cols] = a
        return blob

    return {
        "bhot": build(_BLOB_HOT_LAYOUT, BLOB_HOT_OFF, C_HOT),
        "bcold": build(_BLOB_COLD_LAYOUT, BLOB_COLD_OFF, C_COLD),
        "bf": build(_BLOB_F_LAYOUT, BLOB_F_OFF, C_F),
    }


def make_in_maps(x, hidden, weights):
    x = np.asarray(x, np.float32)
    hidden = np.asarray(hidden, np.float32)
    in_maps = []
    for c in range(NCORES):
        b0 = c * BL
        xs = x[:, b0 : b0 + BL, :]
        xtc = np.concatenate(
            [xs.transpose(2, 0, 1).reshape(IN, S * BL),
             np.ones((1, S * BL), np.float32)], axis=0
        )
        bxt = np.zeros((128, C_XT), np.float32)
        bxt[0 : IN + 1, :] = xtc
        y0 = np.ones((H + 1, 2 * (S + 1)), np.float32)
        y0[0:H, 0:BL] = hidden[0, b0 : b0 + BL, :].T
        in_maps.append({
            "bhot": weights["bhot"],
            "bcold": weights["bcold"],
            "bf": weights["bf"],
            "bxt": bxt,
            "y0": y0,
        })
    return in_maps


def postprocess(results):
    outs = []
    for r in results:
        a = r["out"].reshape(S * S, BL, 2)
        outs.append(np.ascontiguousarray(a))
    return np.concatenate(outs, axis=1)


_NC_CACHE = {}


def get_nc():
    if "nc" not in _NC_CACHE:
        _NC_CACHE["nc"] = build_nc()
    return _NC_CACHE["nc"]


LAST_RESULTS = None


def kernel(x, hidden, W_ih, W_hh, b_ih, b_hh, W1, b1, W2, b2, W3, b3, Wt, bt,
           _run_kwargs=None):
    global LAST_RESULTS
    weights = prep_weights(W_ih, W_hh, b_ih, b_hh, W1, b1, W2, b2, W3, b3, Wt, bt)
    in_maps = make_in_maps(x, hidden, weights)
    nc = get_nc()
    res = run_bass_kernel_spmd(
        nc, in_maps, core_ids=list(range(NCORES)), **(_run_kwargs or {})
    )
    LAST_RESULTS = res
    return postprocess(res.results)
